# revision 19
# baseline (speedup 1.0000x reference)
"""BitMoE FFN (8 experts, top-2, capacity 640) on 8 TRN2 NeuronCores.

Expert-parallel: core i owns expert i (ternary weights quantized on device),
router replicated. Per core: f32 router logits on PE -> top-2 + capacity
(exclusive cumsum in slot order) -> compact token list via one-hot matmuls ->
dma_gather of x rows -> exact-integer fp8/bf16 matmuls for gate/up ->
silu*u -> per-token top-55% |h| threshold (binary search with fused
abs+compare+count) -> masked int8 quant -> ternary down matmul -> combine
scale -> dma_scatter_add into this core's partial output. Host sums the 8
partial outputs.
"""
import os
import sys
import numpy as np

sys.path.insert(0, "/opt/trn_rl_repo")

B, T, D = 4, 1024, 768
BT = B * T
FFN = 3072
NE = 8
TOPK = 2
CAP = 640
KTH = 1690
NT = CAP // 128       # 5
DC = D // 128         # 6
FC = FFN // 512       # 6
KC = FFN // 128       # 24
TT32 = BT // 128      # 32
MAGIC = 12582912.0    # 2**23 + 2**22
SEARCH_ITERS = int(os.environ.get("KB_SEARCH_ITERS", "16"))


def build(sim_sigmoid=False):
    import concourse.bacc as bacc
    import concourse.mybir as mybir
    import concourse.bass_isa as bass_isa
    from concourse import tile
    from concourse import bass

    f32 = mybir.dt.float32
    bf16 = mybir.dt.bfloat16
    fp8 = mybir.dt.float8e4
    i16 = mybir.dt.int16
    i32 = mybir.dt.int32
    A = mybir.AluOpType
    AF = mybir.ActivationFunctionType
    X = mybir.AxisListType.X

    nc = bacc.Bacc(trn_type="TRN2")

    x_d = nc.dram_tensor("x", [BT, D], f32, kind="ExternalInput")
    xT_d = nc.dram_tensor("xT", [D, BT], f32, kind="ExternalInput")
    gwT_d = nc.dram_tensor("gwT", [D, FFN], f32, kind="ExternalInput")
    uwT_d = nc.dram_tensor("uwT", [D, FFN], f32, kind="ExternalInput")
    dwT_d = nc.dram_tensor("dwT", [FFN, D], f32, kind="ExternalInput")
    rw_d = nc.dram_tensor("rw", [NE, D], f32, kind="ExternalInput")
    eid_d = nc.dram_tensor("eid", [1, 1], f32, kind="ExternalInput")
    out_d = nc.dram_tensor("out", [BT, D], f32, kind="ExternalOutput")
    aux_d = nc.dram_tensor("aux", [1, 1], f32, kind="ExternalOutput")
    li_scratch = nc.dram_tensor("li_scr", [1, 640], i16, kind="Internal")
    dbg_lrow = nc.dram_tensor("dbg_lrow", [5, 640], f32, kind="Internal")
    dbg_posm = nc.dram_tensor("dbg_posm", [128, TT32], f32, kind="Internal")
    dbg_cws = nc.dram_tensor("dbg_cws", [128, TT32], f32, kind="Internal")
    dbg_lall = nc.dram_tensor("dbg_lall", [128, TT32 * 8], f32, kind="Internal")
    dbg_vals = nc.dram_tensor("dbg_vals", [128, TT32 * 5], f32, kind="Internal")
    cw_scratch = nc.dram_tensor("cw_scr", [1, 640], f32, kind="Internal")

    iden8 = nc.inline_tensor(np.eye(8, dtype=np.float32), name="iden8")
    lt_np = np.fromfunction(lambda k, m: (k < m), (128, 128)).astype(np.float32)
    ltc = nc.inline_tensor(lt_np, name="ltc")
    ones128 = nc.inline_tensor(np.ones((128, 128), np.float32), name="ones128")
    onescol = nc.inline_tensor(np.ones((128, 1), np.float32), name="onescol")
    iota8 = nc.inline_tensor(
        np.tile(np.arange(8, dtype=np.float32)[None, :], (128, 1)), name="iota8")
    iota640 = nc.inline_tensor(
        np.tile(np.arange(640, dtype=np.float32)[None, :], (128, 1)), name="iota640")
    tok_hi = nc.inline_tensor(
        np.fromfunction(lambda p, t: np.floor((t * 128 + p) / 64), (128, TT32)
                        ).astype(np.float32), name="tok_hi")
    tok_lo = nc.inline_tensor(
        np.fromfunction(lambda p, t: (t * 128 + p) % 64, (128, TT32)
                        ).astype(np.float32), name="tok_lo")

    with tile.TileContext(nc) as tc:
        with (
            tc.tile_pool(name="consts", bufs=1) as pconst,
            tc.tile_pool(name="wq", bufs=1) as pwq,
            tc.tile_pool(name="router", bufs=1) as prt,
            tc.tile_pool(name="xts", bufs=4) as pxt,
            tc.tile_pool(name="wrk", bufs=1) as pffn,
            tc.tile_pool(name="hbuf", bufs=1) as phb,
            tc.tile_pool(name="small", bufs=1) as psm,
        ):
            # ---------- constants ----------
            iden8_t = pconst.tile([8, 8], f32)
            nc.sync.dma_start(iden8_t[:], iden8[:])
            ltf = pconst.tile([128, 128], f32, tag="cf128")
            nc.sync.dma_start(ltf[:], ltc[:])
            lt_bf = pconst.tile([128, 128], bf16)
            nc.vector.tensor_copy(lt_bf[:], ltf[:])
            onesf2 = pconst.tile([128, 128], f32, tag="cf128b")
            nc.sync.dma_start(onesf2[:], ones128[:])
            ones_bf = pconst.tile([128, 128], bf16)
            nc.vector.tensor_copy(ones_bf[:], onesf2[:])
            onescol_f = pconst.tile([128, 1], f32)
            nc.sync.dma_start(onescol_f[:], onescol[:])
            iota8_t = pconst.tile([128, 8], f32)
            nc.sync.dma_start(iota8_t[:], iota8[:])
            iota640_t = pconst.tile([128, 640], f32)
            nc.sync.dma_start(iota640_t[:], iota640[:])
            thi_t = pconst.tile([128, TT32], f32)
            nc.sync.dma_start(thi_t[:], tok_hi[:])
            tlo_t = pconst.tile([128, TT32], f32)
            nc.sync.dma_start(tlo_t[:], tok_lo[:])
            eid_t = pconst.tile([1, 1], f32)
            nc.sync.dma_start(eid_t[:], eid_d[:])
            eid_b = pconst.tile([128, 1], f32)
            nc.gpsimd.partition_broadcast(eid_b[:], eid_t[:])
            eqe = pconst.tile([128, 8], f32)
            nc.vector.tensor_scalar(eqe[:], iota8_t[:], eid_b[:, 0:1], None, A.is_equal)
            zcol = pconst.tile([128, 1], f32)
            nc.vector.memset(zcol[:], 0.0)

            with tc.tile_pool(name="rpsum", bufs=1,
                              space=bass.MemorySpace.PSUM) as ppr:
                # ---------- router weight int8 quant ----------
                rw_t = prt.tile([8, D], f32)
                nc.sync.dma_start(rw_t[:], rw_d[:])
                rwabs = prt.tile([8, D], f32)
                nc.scalar.activation(rwabs[:], rw_t[:], AF.Abs)
                rcolmax = prt.tile([8, D], f32)
                nc.gpsimd.partition_all_reduce(
                    rcolmax[:], rwabs[:], channels=8,
                    reduce_op=bass_isa.ReduceOp.max)
                ramax = prt.tile([8, 1], f32)
                nc.vector.tensor_reduce(ramax[:], rcolmax[:], axis=X, op=A.max)
                rrec = prt.tile([8, 1], f32)
                nc.vector.reciprocal(rrec[:], ramax[:])
                rs8 = prt.tile([8, 1], f32)
                nc.vector.tensor_scalar_mul(rs8[:], rrec[:], 127.0)
                rq1 = prt.tile([8, D], f32, tag="rwabs")
                nc.vector.tensor_scalar(rq1[:], rw_t[:], rs8[:, 0:1], MAGIC,
                                        A.mult, A.add)
                rsc = prt.tile([8, 1], f32)
                nc.vector.tensor_scalar_mul(rsc[:], ramax[:], 1.0 / 127.0)
                rwq = prt.tile([8, D], f32, tag="rwt")
                nc.vector.tensor_scalar(rwq[:], rq1[:], MAGIC, rsc[:, 0:1],
                                        A.subtract, A.mult)
                rwqT = prt.tile([128, DC, 8], f32)
                for c in range(DC):
                    tp = ppr.tile([128, 8], f32, tag="tp")
                    nc.tensor.transpose(tp[:], rwq[:, c * 128:(c + 1) * 128],
                                        iden8_t[:])
                    nc.scalar.copy(rwqT[:, c, :], tp[:])

                # ---------- router logits ----------
                l_ps = ppr.tile([128, TT32, 8], f32, tag="lps")
                for t in range(TT32):
                    for c in range(DC):
                        xTc = pxt.tile([128, 128], f32, tag="xtc")
                        nc.sync.dma_start(
                            xTc[:], xT_d[c * 128:(c + 1) * 128,
                                         t * 128:(t + 1) * 128])
                        nc.tensor.matmul(
                            l_ps[:, t, :], xTc[:], rwqT[:, c, :],
                            start=(c == 0), stop=(c == DC - 1))
                l_all = prt.tile([128, TT32, 8], f32)
                nc.vector.tensor_copy(l_all[:], l_ps[:])

                # ---------- top-2 ----------
                S3 = (128, TT32, 8)
                m1 = prt.tile([128, TT32], f32)
                nc.vector.tensor_reduce(m1[:], l_all[:], axis=X, op=A.max)
                eq1 = prt.tile([128, TT32, 8], f32)
                nc.vector.tensor_tensor(eq1[:], l_all[:],
                                        m1[:].broadcast_to(S3), A.is_equal)
                lm = prt.tile([128, TT32, 8], f32, tag="lm")
                nc.vector.scalar_tensor_tensor(lm[:], eq1[:], -1e30, l_all[:],
                                               A.mult, A.add)
                m2 = prt.tile([128, TT32], f32)
                nc.vector.tensor_reduce(m2[:], lm[:], axis=X, op=A.max)
                eq2 = prt.tile([128, TT32, 8], f32)
                nc.vector.tensor_tensor(eq2[:], lm[:],
                                        m2[:].broadcast_to(S3), A.is_equal)
                dlt = prt.tile([128, TT32], f32)
                nc.vector.tensor_sub(dlt[:], m1[:], m2[:])
                w1 = prt.tile([128, TT32], f32)
                nc.scalar.activation(w1[:], dlt[:], AF.Sigmoid)
                w2 = prt.tile([128, TT32], f32)
                nc.vector.tensor_scalar(w2[:], w1[:], -1.0, 1.0, A.mult, A.add)

                # ---------- softmax (aux) ----------
                dsub = prt.tile([128, TT32, 8], f32, tag="dsub")
                nc.vector.tensor_tensor(dsub[:], l_all[:],
                                        m1[:].broadcast_to(S3), A.subtract)
                ex = prt.tile([128, TT32, 8], f32, tag="lm")
                nc.scalar.activation(ex[:], dsub[:], AF.Exp)
                exs = prt.tile([128, TT32], f32)
                nc.vector.tensor_reduce(exs[:], ex[:], axis=X, op=A.add)
                exr = prt.tile([128, TT32], f32)
                nc.vector.reciprocal(exr[:], exs[:])
                probs = prt.tile([128, TT32, 8], f32, tag="dsub2")
                nc.vector.tensor_tensor(probs[:], ex[:],
                                        exr[:].broadcast_to(S3), A.mult)
                cnt2 = prt.tile([128, TT32, 8], f32)
                nc.vector.tensor_add(cnt2[:], eq1[:], eq2[:])

                # ---------- capacity cumsum ----------
                cnt_bf = prt.tile([128, TT32, 8], bf16)
                nc.vector.tensor_copy(cnt_bf[:], cnt2[:])
                pref_ps = ppr.tile([128, TT32, 8], f32, tag="prefps")
                tot_ps = ppr.tile([128, TT32, 8], f32, tag="totps")
                for t in range(TT32):
                    nc.tensor.matmul(pref_ps[:, t, :], lt_bf[:], cnt_bf[:, t, :])
                    nc.tensor.matmul(tot_ps[:, t, :], ones_bf[:], cnt_bf[:, t, :])
                tot_sb = prt.tile([128, TT32, 8], f32, tag="dsub")
                nc.vector.tensor_copy(tot_sb[:], tot_ps[:])
                tinc = prt.tile([128, TT32, 8], f32, tag="lm")
                for e in range(8):
                    nc.vector.tensor_tensor_scan(
                        tinc[:, :, e], tot_sb[:, :, e],
                        zcol[:, 0:1].broadcast_to((128, TT32)), 0.0,
                        A.add, A.add)
                pos_all = prt.tile([128, TT32, 8], f32)
                nc.vector.tensor_sub(pos_all[:], tinc[:], tot_sb[:])
                nc.vector.tensor_add(pos_all[:], pos_all[:], pref_ps[:])

                # ---------- our-expert masks/positions ----------
                tmp8 = prt.tile([128, TT32, 8], f32, tag="dsub")
                eqe_bc = eqe[:].broadcast_to((128, 8, TT32)).rearrange(
                    "p e t -> p t e")
                nc.vector.tensor_tensor(tmp8[:], eq1[:], eqe_bc, A.mult)
                mk1 = prt.tile([128, TT32], f32)
                nc.vector.tensor_reduce(mk1[:], tmp8[:], axis=X, op=A.add)
                nc.vector.tensor_tensor(tmp8[:], eq2[:], eqe_bc, A.mult)
                mk2 = prt.tile([128, TT32], f32)
                nc.vector.tensor_reduce(mk2[:], tmp8[:], axis=X, op=A.add)
                nc.vector.tensor_tensor(tmp8[:], pos_all[:], eqe_bc, A.mult)
                pos0 = prt.tile([128, TT32], f32)
                nc.vector.tensor_reduce(pos0[:], tmp8[:], axis=X, op=A.add)
                kle = prt.tile([128, TT32], f32)
                nc.vector.tensor_scalar(kle[:], pos0[:], float(CAP - 1), None,
                                        A.is_le)
                v1 = prt.tile([128, TT32], f32)
                nc.vector.tensor_tensor(v1[:], mk1[:], kle[:], A.mult)
                v2 = prt.tile([128, TT32], f32)
                nc.vector.tensor_tensor(v2[:], mk2[:], kle[:], A.mult)
                vmask = prt.tile([128, TT32], f32)
                nc.vector.tensor_add(vmask[:], v1[:], v2[:])
                cwa = prt.tile([128, TT32], f32)
                nc.vector.tensor_tensor(cwa[:], v1[:], w1[:], A.mult)
                cwb = prt.tile([128, TT32], f32)
                nc.vector.tensor_tensor(cwb[:], v2[:], w2[:], A.mult)
                cwsel = prt.tile([128, TT32], f32)
                nc.vector.tensor_add(cwsel[:], cwa[:], cwb[:])
                posm = prt.tile([128, TT32], f32)
                t1m = prt.tile([128, TT32], f32, tag="cwa")
                nc.vector.tensor_tensor(t1m[:], pos0[:], vmask[:], A.mult)
                t2m = prt.tile([128, TT32], f32, tag="cwb")
                nc.vector.tensor_scalar(t2m[:], vmask[:], 1.0, 1e6,
                                        A.subtract, A.mult)
                nc.vector.tensor_add(posm[:], t1m[:], t2m[:])

                # ---------- values lhsT [128, 32, 5] ----------
                vals = prt.tile([128, TT32, 5], bf16)
                nc.vector.tensor_copy(vals[:, :, 0], thi_t[:])
                nc.vector.tensor_copy(vals[:, :, 1], tlo_t[:])
                nc.vector.tensor_copy(vals[:, :, 2], cwsel[:])
                cwlo = prt.tile([128, TT32], f32, tag="cwa")
                nc.vector.tensor_tensor(cwlo[:], cwsel[:], vals[:, :, 2],
                                        A.subtract)
                nc.vector.tensor_copy(vals[:, :, 3], cwlo[:])
                nc.vector.tensor_copy(vals[:, :, 4], vmask[:])

                # ---------- one-hot scatter -> list [5, 640] ----------
                list_ps0 = ppr.tile([5, 320], f32, tag="listps0")
                list_ps1 = ppr.tile([5, 320], f32, tag="listps1")
                list_ps = [list_ps0, list_ps1]
                for t in range(TT32):
                    oh = prt.tile([128, 640], bf16, tag="oh")
                    nc.vector.tensor_scalar(oh[:], iota640_t[:],
                                            posm[:, t:t + 1], None, A.is_equal)
                    for j in range(2):
                        nc.tensor.matmul(
                            list_ps[j][:], vals[:, t, :],
                            oh[:, j * 320:(j + 1) * 320],
                            start=(t == 0), stop=(t == TT32 - 1))

                # ---------- finalize list ----------
                lrow = psm.tile([5, 640], f32)
                nc.vector.tensor_copy(lrow[:, 0:320], list_ps[0][:])
                nc.vector.tensor_copy(lrow[:, 320:640], list_ps[1][:])
                nc.sync.dma_start(dbg_lrow[:], lrow[:])
                nc.sync.dma_start(dbg_posm[:], posm[:])
                nc.sync.dma_start(dbg_cws[:], cwsel[:])
                nc.sync.dma_start(dbg_lall[:], l_all[:].rearrange("p t e -> p (t e)"))
                valsf = prt.tile([128, TT32, 5], f32, tag="dsub2")
                nc.vector.tensor_copy(valsf[:], vals[:])
                nc.sync.dma_start(dbg_vals[:], valsf[:].rearrange("p t e -> p (t e)"))
                lr_a = psm.tile([1, 640], f32, tag="lra")
                nc.sync.dma_start(lr_a[:], lrow[1:2, :])
                tokf = psm.tile([1, 640], f32, tag="tokf")
                nc.vector.scalar_tensor_tensor(
                    tokf[:], lrow[0:1, :], 64.0, lr_a[:], A.mult, A.add)
                lr_a2 = psm.tile([1, 640], f32, tag="lra")
                nc.sync.dma_start(lr_a2[:], lrow[2:3, :])
                lr_b = psm.tile([1, 640], f32, tag="lrb")
                nc.sync.dma_start(lr_b[:], lrow[3:4, :])
                cwf = psm.tile([1, 640], f32, tag="cwf")
                nc.vector.tensor_tensor(cwf[:], lr_a2[:], lr_b[:], A.add)
                lr_a3 = psm.tile([1, 640], f32, tag="lra")
                nc.sync.dma_start(lr_a3[:], lrow[4:5, :])
                n0f = psm.tile([1, 1], f32)
                nc.vector.tensor_reduce(n0f[:], lr_a3[:], axis=X, op=A.add)
                n0i = psm.tile([1, 1], i32)
                nc.vector.tensor_copy(n0i[:], n0f[:])
                mle = psm.tile([1, 640], f32, tag="lrb")
                nc.vector.tensor_scalar(mle[:], iota640_t[0:1, :],
                                        n0f[0:1, 0:1], None, A.is_lt)
                lfin = psm.tile([1, 640], f32, tag="lra")
                nc.vector.scalar_tensor_tensor(
                    lfin[:], tokf[:], 1.0, mle[:], A.add, A.mult)
                nc.vector.tensor_scalar(lfin[:], lfin[:], 1.0, None, A.subtract)
                li16 = psm.tile([1, 640], i16)
                nc.vector.tensor_copy(li16[:], lfin[:])
                nc.sync.dma_start(li_scratch[:], li16[:])
                nc.sync.dma_start(cw_scratch[:], cwf[:])
                idx16 = psm.tile([128, 40], i16)
                for r in range(8):
                    nc.sync.dma_start(
                        idx16[16 * r:16 * (r + 1), :],
                        li_scratch[:].rearrange("a (c p) -> (a p) c", p=16))
                cw_tl = psm.tile([128, NT], f32)
                nc.sync.dma_start(
                    cw_tl[:], cw_scratch[:].rearrange("a (b p) -> (a p) b", p=128))

                # ---------- aux ----------
                cnt_sum = ppr.tile([1, TT32, 8], f32, tag="auxc")
                prob_sum = ppr.tile([1, TT32, 8], f32, tag="auxp")
                nc.tensor.matmul(cnt_sum[:].rearrange("a b c -> a (b c)"),
                                 onescol_f[:],
                                 cnt2[:].rearrange("p t e -> p (t e)"))
                nc.tensor.matmul(prob_sum[:].rearrange("a b c -> a (b c)"),
                                 onescol_f[:],
                                 probs[:].rearrange("p t e -> p (t e)"))
                cnt_e = psm.tile([1, 8], f32)
                nc.vector.tensor_reduce(
                    cnt_e[:], cnt_sum[:].rearrange("a t e -> a e t"),
                    axis=X, op=A.add)
                prob_e = psm.tile([1, 8], f32)
                nc.vector.tensor_reduce(
                    prob_e[:], prob_sum[:].rearrange("a t e -> a e t"),
                    axis=X, op=A.add)
                fp = psm.tile([1, 8], f32)
                nc.vector.tensor_tensor(fp[:], cnt_e[:], prob_e[:], A.mult)
                auxv = psm.tile([1, 1], f32)
                nc.vector.tensor_reduce(auxv[:], fp[:], axis=X, op=A.add)
                nc.vector.tensor_scalar_mul(auxv[:], auxv[:],
                                            float(NE) / (BT * TOPK * BT))
                nc.sync.dma_start(aux_d[:], auxv[:])

            # ---------- weights + FFN ----------
            with tc.tile_pool(name="fpsum", bufs=2,
                              space=bass.MemorySpace.PSUM) as ppf, \
                 tc.tile_pool(name="fpsum1", bufs=1,
                              space=bass.MemorySpace.PSUM) as ppy, \
                 tc.tile_pool(name="wstream", bufs=2) as pws:

                def quant_weight(w_d, rows, cols, out_tile, out_dt):
                    nchunks = rows // 128
                    NSP = 4 if cols >= 3072 else 2
                    hcol = cols // NSP
                    parts = psm.tile([128, NSP * nchunks], f32, tag="wp" + w_d.name)
                    for ck in range(NSP * nchunks):
                        wc = pws.tile([128, hcol], f32, tag="wst")
                        nc.sync.dma_start(
                            wc[:], w_d[(ck // NSP) * 128:(ck // NSP + 1) * 128,
                                       (ck % NSP) * hcol:(ck % NSP + 1) * hcol])
                        nc.scalar.activation(
                            wc[:], wc[:], AF.Abs,
                            accum_out=parts[:, ck:ck + 1])
                    prow = psm.tile([128, 1], f32, tag="pr" + w_d.name)
                    nc.vector.tensor_reduce(prow[:], parts[:], axis=X, op=A.add)
                    tot2 = ppy.tile([1, 1], f32, tag="wtot")
                    nc.tensor.matmul(tot2[:], onescol_f[:], prow[:])
                    rec = psm.tile([1, 1], f32, tag="rc" + w_d.name)
                    nc.vector.reciprocal(rec[:], tot2[:])
                    rs = psm.tile([1, 1], f32, tag="rs" + w_d.name)
                    nc.vector.tensor_scalar_mul(rs[:], rec[:],
                                                float(rows * cols))
                    rs_b = psm.tile([128, 1], f32, tag="rb" + w_d.name)
                    nc.gpsimd.partition_broadcast(rs_b[:], rs[:])
                    sca = psm.tile([1, 1], f32, tag="sc" + w_d.name)
                    nc.vector.tensor_scalar_mul(sca[:], tot2[:],
                                                1.0 / (rows * cols))
                    s_b = psm.tile([128, 1], f32, tag="sb" + w_d.name)
                    nc.gpsimd.partition_broadcast(s_b[:], sca[:])
                    for ck in range(NSP * nchunks):
                        wc = pws.tile([128, hcol], f32, tag="wst")
                        nc.sync.dma_start(
                            wc[:], w_d[(ck // NSP) * 128:(ck // NSP + 1) * 128,
                                       (ck % NSP) * hcol:(ck % NSP + 1) * hcol])
                        q1 = pffn.tile([128, hcol], f32, tag="wk_a")
                        nc.vector.tensor_scalar(q1[:], wc[:], rs_b[:, 0:1],
                                                MAGIC, A.mult, A.add)
                        q2 = pffn.tile([128, hcol], f32, tag="wk_b")
                        nc.vector.tensor_scalar(q2[:], q1[:], MAGIC - 1.0, 2.0,
                                                A.subtract, A.min)
                        nc.vector.tensor_scalar(
                            out_tile[:, ck // NSP,
                                     (ck % NSP) * hcol:(ck % NSP + 1) * hcol],
                            q2[:], 0.0, None, A.max)
                    return s_b

                gq = pwq.tile([128, DC, FFN], fp8, tag="gq")
                uq = pwq.tile([128, DC, FFN], fp8, tag="uq")
                dq = pwq.tile([128, KC, D], bf16, tag="dq")
                sg_b = quant_weight(gwT_d, D, FFN, gq, fp8)
                su_b = quant_weight(uwT_d, D, FFN, uq, fp8)
                sd_b = quant_weight(dwT_d, FFN, D, dq, bf16)

                # ---------- gather + int4 quant ----------
                n0_reg = nc.gpsimd.alloc_register("n0reg")
                xg = phb.tile([128, NT, D], f32, tag="xgy")
                dgsem = nc.alloc_semaphore("dgsem")
                with tc.tile_critical():
                    nc.gpsimd.load(n0_reg, n0i[0:1, 0:1])
                    nc.gpsimd.dma_gather(
                        xg[:], x_d[:, :], idx16[:], 640, n0_reg, D
                    ).then_inc(dgsem, 16)
                    nc.gpsimd.wait_ge(dgsem, 16)
                amax5 = psm.tile([128, NT], f32)
                nc.vector.tensor_reduce(amax5[:], xg[:], axis=X, op=A.max,
                                        apply_absolute_value=True)
                r5 = psm.tile([128, NT], f32)
                nc.vector.reciprocal(r5[:], amax5[:])
                s7 = psm.tile([128, NT], f32)
                nc.vector.tensor_scalar_mul(s7[:], r5[:], 7.0)
                xq = pffn.tile([128, NT, D], bf16, tag="xq")
                sumxq = psm.tile([128, NT], f32)
                for t in range(NT):
                    xv1 = pffn.tile([128, D], f32, tag="xv1")
                    nc.vector.tensor_scalar(xv1[:], xg[:, t, :], s7[:, t:t + 1],
                                            MAGIC, A.mult, A.add)
                    nc.vector.tensor_scalar(xq[:, t, :], xv1[:], MAGIC, 0.0,
                                            A.subtract, A.add,
                                            accum_out=sumxq[:, t:t + 1])
                xqT = pffn.tile([128, NT, DC, 128], fp8, tag="xqT")
                for t in range(NT):
                    for c in range(DC):
                        xtt = pffn.tile([128, 128], bf16, tag="xtt")
                        nc.sync.dma_start_transpose(
                            xtt[:], xq[:, t, c * 128:(c + 1) * 128])
                        nc.vector.tensor_copy(xqT[:, t, c, :], xtt[:])

                gsc = psm.tile([128, NT], f32)
                nc.vector.tensor_tensor(gsc[:], amax5[:],
                                        sg_b[:, 0:1].broadcast_to((128, NT)),
                                        A.mult)
                nc.vector.tensor_scalar_mul(gsc[:], gsc[:], 1.0 / 7.0)
                gbias = psm.tile([128, NT], f32)
                nc.vector.tensor_tensor(gbias[:], sumxq[:], gsc[:], A.mult)
                nc.vector.tensor_scalar_mul(gbias[:], gbias[:], -1.0)
                usc = psm.tile([128, NT], f32)
                nc.vector.tensor_tensor(usc[:], amax5[:],
                                        su_b[:, 0:1].broadcast_to((128, NT)),
                                        A.mult)
                nc.vector.tensor_scalar_mul(usc[:], usc[:], 1.0 / 7.0)

                yout = phb.tile([128, NT, D], f32, tag="xgy")
                act_fn = AF.Sigmoid if sim_sigmoid else AF.Silu

                for t in range(NT):
                    htil = phb.tile([128, FFN], f32, tag="htil")
                    for fc in range(FC):
                        g_ps = ppf.tile([128, 512], f32, tag="gps")
                        u_ps = ppf.tile([128, 512], f32, tag="ups")
                        for c in range(DC):
                            nc.tensor.matmul(
                                g_ps[:], xqT[:, t, c, :],
                                gq[:, c, fc * 512:(fc + 1) * 512],
                                start=(c == 0), stop=(c == DC - 1))
                        for c in range(DC):
                            nc.tensor.matmul(
                                u_ps[:], xqT[:, t, c, :],
                                uq[:, c, fc * 512:(fc + 1) * 512],
                                start=(c == 0), stop=(c == DC - 1))
                        sgl = pffn.tile([128, 512], f32, tag="sgl")
                        nc.scalar.activation(
                            sgl[:], g_ps[:], act_fn,
                            bias=gbias[:, t:t + 1], scale=gsc[:, t:t + 1])
                        ucc = pffn.tile([128, 512], f32, tag="ucc")
                        nc.vector.tensor_scalar(
                            ucc[:], u_ps[:], sumxq[:, t:t + 1], None,
                            A.subtract)
                        nc.vector.tensor_tensor(
                            htil[:, fc * 512:(fc + 1) * 512], sgl[:], ucc[:],
                            A.mult)
                    amaxh = psm.tile([128, 1], f32, tag="amaxh")
                    nc.vector.tensor_reduce(amaxh[:], htil[:], axis=X, op=A.max,
                                            apply_absolute_value=True)
                    lo = psm.tile([128, 1], f32, tag="lo")
                    nc.vector.memset(lo[:], 0.0)
                    hi = psm.tile([128, 1], f32, tag="hi")
                    nc.vector.tensor_copy(hi[:], amaxh[:])
                    mid = psm.tile([128, 1], f32, tag="mid")
                    nmid = psm.tile([128, 1], f32, tag="nmid")
                    cnta = psm.tile([128, 1], f32, tag="cnta")
                    cntb = psm.tile([128, 1], f32, tag="cntb")
                    cntt = psm.tile([128, 1], f32, tag="cntt")
                    junk = pffn.tile([128, FFN], bf16, tag="hq")
                    for it in range(SEARCH_ITERS):
                        nc.vector.tensor_add(mid[:], lo[:], hi[:])
                        nc.vector.tensor_scalar_mul(mid[:], mid[:], 0.5)
                        nc.vector.tensor_scalar_mul(nmid[:], mid[:], -1.0)
                        nc.vector.tensor_scalar(
                            junk[:], htil[:], mid[:, 0:1], 0.0,
                            A.is_ge, A.add, accum_out=cnta[:])
                        nc.vector.tensor_scalar(
                            junk[:], htil[:], nmid[:, 0:1], 0.0,
                            A.is_le, A.add, accum_out=cntb[:])
                        nc.vector.tensor_add(cntt[:], cnta[:], cntb[:])
                        ge = psm.tile([128, 1], mybir.dt.uint8, tag="ge")
                        nc.vector.tensor_scalar(ge[:], cntt[:], float(KTH),
                                                None, A.is_ge)
                        nc.vector.copy_predicated(lo[:], ge[:], mid[:])
                        gei = psm.tile([128, 1], mybir.dt.uint8, tag="gei")
                        nc.vector.tensor_scalar(gei[:], cntt[:], float(KTH),
                                                None, A.is_lt)
                        nc.vector.copy_predicated(hi[:], gei[:], mid[:])
                    s8r = psm.tile([128, 1], f32, tag="s8r")
                    nc.vector.reciprocal(s8r[:], amaxh[:])
                    s8 = psm.tile([128, 1], f32, tag="s8")
                    nc.vector.tensor_scalar_mul(s8[:], s8r[:], 127.0)
                    nlo = psm.tile([128, 1], f32, tag="nmid")
                    nc.vector.tensor_scalar_mul(nlo[:], lo[:], -1.0)
                    mska = pffn.tile([128, FFN], bf16, tag="hqT")
                    nc.vector.tensor_scalar(mska[:], htil[:], lo[:, 0:1], None,
                                            A.is_ge)
                    msk = pffn.tile([128, FFN], bf16, tag="msk")
                    nc.vector.tensor_scalar(msk[:], htil[:], nlo[:, 0:1], 1.0,
                                            A.is_le, A.mult)
                    nc.vector.tensor_add(msk[:], msk[:], mska[:])
                    nc.vector.tensor_scalar(htil[:], htil[:], s8[:, 0:1], MAGIC,
                                            A.mult, A.add)
                    hq = pffn.tile([128, FFN], bf16, tag="hq")
                    sumq = psm.tile([128, 1], f32, tag="sumq")
                    nc.vector.scalar_tensor_tensor(
                        hq[:], htil[:], MAGIC, msk[:], A.subtract, A.mult,
                        accum_out=sumq[:])
                    hqT = pffn.tile([128, KC, 128], bf16, tag="hqT")
                    for c in range(KC):
                        nc.sync.dma_start_transpose(
                            hqT[:, c, :], hq[:, c * 128:(c + 1) * 128])
                    y_ps0 = ppy.tile([128, 384], f32, tag="yps0")
                    y_ps1 = ppy.tile([128, 384], f32, tag="yps1")
                    y_ps = [y_ps0, y_ps1]
                    for j in range(2):
                        for c in range(KC):
                            nc.tensor.matmul(
                                y_ps[j][:], hqT[:, c, :],
                                dq[:, c, j * 384:(j + 1) * 384],
                                start=(c == 0), stop=(c == KC - 1))
                    fsc = psm.tile([128, 1], f32, tag="fsc")
                    nc.vector.tensor_tensor(fsc[:], amaxh[:], usc[:, t:t + 1],
                                            A.mult)
                    nc.vector.tensor_tensor(fsc[:], fsc[:], sd_b[:, 0:1],
                                            A.mult)
                    nc.vector.tensor_tensor(fsc[:], fsc[:], cw_tl[:, t:t + 1],
                                            A.mult)
                    nc.vector.tensor_scalar_mul(fsc[:], fsc[:], 1.0 / 127.0)
                    for j in range(2):
                        nc.vector.tensor_scalar(
                            yout[:, t, j * 384:(j + 1) * 384], y_ps[j][:],
                            sumq[:, 0:1], fsc[:, 0:1], A.subtract, A.mult)

                scsem = nc.alloc_semaphore("scsem")
                with tc.tile_critical():
                    nc.gpsimd.dma_scatter_add(
                        out_d[:, :], yout[:], idx16[:], 640, n0_reg, D
                    ).then_inc(scsem, 16)
                    nc.gpsimd.wait_ge(scsem, 16)

    nc.compile()
    return nc


_NC_CACHE = {}


def _get_nc(sim_sigmoid=False):
    key = bool(sim_sigmoid)
    if key not in _NC_CACHE:
        _NC_CACHE[key] = build(sim_sigmoid=key)
    return _NC_CACHE[key]


def make_in_maps(x, gate_w, up_w, down_w, router_w):
    x2 = np.ascontiguousarray(np.asarray(x).reshape(BT, D).astype(np.float32))
    xT = np.ascontiguousarray(x2.T)
    rw = np.ascontiguousarray(np.asarray(router_w).astype(np.float32))
    in_maps = []
    for e in range(NE):
        in_maps.append({
            "x": x2,
            "xT": xT,
            "gwT": np.ascontiguousarray(np.asarray(gate_w[e]).T.astype(np.float32)),
            "uwT": np.ascontiguousarray(np.asarray(up_w[e]).T.astype(np.float32)),
            "dwT": np.ascontiguousarray(np.asarray(down_w[e]).T.astype(np.float32)),
            "rw": rw,
            "eid": np.array([[float(e)]], dtype=np.float32),
        })
    return in_maps


def kernel(x, gate_w, up_w, down_w, router_w):
    from concourse.bass_utils import run_bass_kernel_spmd

    nc = _get_nc(sim_sigmoid=False)
    in_maps = make_in_maps(x, gate_w, up_w, down_w, router_w)
    res = run_bass_kernel_spmd(nc, in_maps, core_ids=list(range(NE)))
    out = np.zeros((BT, D), np.float32)
    for e in range(NE):
        out += res.results[e]["out"]
    aux = np.float32(res.results[0]["aux"][0, 0])
    return out.reshape(B, T, D), aux


# revision 20
# speedup vs baseline: 1.0945x; 1.0945x over previous
"""BitMoE FFN (8 experts, top-2, capacity 640) on 8 TRN2 NeuronCores.

Expert-parallel: core i owns expert i (ternary weights quantized on device),
router replicated. Per core: f32 router logits on PE -> top-2 + capacity
(exclusive cumsum in slot order) -> compact token list via one-hot matmuls ->
dma_gather of x rows -> exact-integer fp8/bf16 matmuls for gate/up ->
silu*u -> per-token top-55% |h| threshold (binary search with fused
abs+compare+count) -> masked int8 quant -> ternary down matmul -> combine
scale -> dma_scatter_add into this core's partial output. Host sums the 8
partial outputs.
"""
import os
import sys
import numpy as np

sys.path.insert(0, "/opt/trn_rl_repo")

B, T, D = 4, 1024, 768
BT = B * T
FFN = 3072
NE = 8
TOPK = 2
CAP = 640
KTH = 1690
NT = CAP // 128       # 5
DC = D // 128         # 6
FC = FFN // 512       # 6
KC = FFN // 128       # 24
TT32 = BT // 128      # 32
MAGIC = 12582912.0    # 2**23 + 2**22
SEARCH_ITERS = int(os.environ.get("KB_SEARCH_ITERS", "13"))


def build(sim_sigmoid=False):
    import concourse.bacc as bacc
    import concourse.mybir as mybir
    import concourse.bass_isa as bass_isa
    from concourse import tile
    from concourse import bass

    f32 = mybir.dt.float32
    bf16 = mybir.dt.bfloat16
    fp8 = mybir.dt.float8e4
    i16 = mybir.dt.int16
    i32 = mybir.dt.int32
    A = mybir.AluOpType
    AF = mybir.ActivationFunctionType
    X = mybir.AxisListType.X

    nc = bacc.Bacc(trn_type="TRN2")

    x_d = nc.dram_tensor("x", [BT, D], f32, kind="ExternalInput")
    xT_d = nc.dram_tensor("xT", [D, BT], f32, kind="ExternalInput")
    gwT_d = nc.dram_tensor("gwT", [D, FFN], f32, kind="ExternalInput")
    uwT_d = nc.dram_tensor("uwT", [D, FFN], f32, kind="ExternalInput")
    dwT_d = nc.dram_tensor("dwT", [FFN, D], f32, kind="ExternalInput")
    rw_d = nc.dram_tensor("rw", [NE, D], f32, kind="ExternalInput")
    eid_d = nc.dram_tensor("eid", [1, 1], f32, kind="ExternalInput")
    out_d = nc.dram_tensor("out", [BT, D], f32, kind="ExternalOutput")
    aux_d = nc.dram_tensor("aux", [1, 1], f32, kind="ExternalOutput")
    li_scratch = nc.dram_tensor("li_scr", [1, 640], i16, kind="Internal")
    dbg_lrow = nc.dram_tensor("dbg_lrow", [5, 640], f32, kind="Internal")
    dbg_posm = nc.dram_tensor("dbg_posm", [128, TT32], f32, kind="Internal")
    dbg_cws = nc.dram_tensor("dbg_cws", [128, TT32], f32, kind="Internal")
    dbg_lall = nc.dram_tensor("dbg_lall", [128, TT32 * 8], f32, kind="Internal")
    dbg_vals = nc.dram_tensor("dbg_vals", [128, TT32 * 5], f32, kind="Internal")
    cw_scratch = nc.dram_tensor("cw_scr", [1, 640], f32, kind="Internal")

    iden8 = nc.inline_tensor(np.eye(8, dtype=np.float32), name="iden8")
    lt_np = np.fromfunction(lambda k, m: (k < m), (128, 128)).astype(np.float32)
    ltc = nc.inline_tensor(lt_np, name="ltc")
    ones128 = nc.inline_tensor(np.ones((128, 128), np.float32), name="ones128")
    onescol = nc.inline_tensor(np.ones((128, 1), np.float32), name="onescol")
    iota8 = nc.inline_tensor(
        np.tile(np.arange(8, dtype=np.float32)[None, :], (128, 1)), name="iota8")
    iota640 = nc.inline_tensor(
        np.tile(np.arange(640, dtype=np.float32)[None, :], (128, 1)), name="iota640")
    tok_hi = nc.inline_tensor(
        np.fromfunction(lambda p, t: np.floor((t * 128 + p) / 64), (128, TT32)
                        ).astype(np.float32), name="tok_hi")
    tok_lo = nc.inline_tensor(
        np.fromfunction(lambda p, t: (t * 128 + p) % 64, (128, TT32)
                        ).astype(np.float32), name="tok_lo")

    with tile.TileContext(nc) as tc:
        with (
            tc.tile_pool(name="consts", bufs=1) as pconst,
            tc.tile_pool(name="wq", bufs=1) as pwq,
            tc.tile_pool(name="router", bufs=1) as prt,
            tc.tile_pool(name="xts", bufs=4) as pxt,
            tc.tile_pool(name="wrk", bufs=1) as pffn,
            tc.tile_pool(name="hbuf", bufs=1) as phb,
            tc.tile_pool(name="small", bufs=1) as psm,
        ):
            # ---------- constants ----------
            iden8_t = pconst.tile([8, 8], f32)
            nc.sync.dma_start(iden8_t[:], iden8[:])
            ltf = pconst.tile([128, 128], f32, tag="cf128")
            nc.sync.dma_start(ltf[:], ltc[:])
            lt_bf = pconst.tile([128, 128], bf16)
            nc.vector.tensor_copy(lt_bf[:], ltf[:])
            onesf2 = pconst.tile([128, 128], f32, tag="cf128b")
            nc.sync.dma_start(onesf2[:], ones128[:])
            ones_bf = pconst.tile([128, 128], bf16)
            nc.vector.tensor_copy(ones_bf[:], onesf2[:])
            onescol_f = pconst.tile([128, 1], f32)
            nc.sync.dma_start(onescol_f[:], onescol[:])
            iota8_t = pconst.tile([128, 8], f32)
            nc.sync.dma_start(iota8_t[:], iota8[:])
            iota640_t = pconst.tile([128, 640], f32)
            nc.sync.dma_start(iota640_t[:], iota640[:])
            thi_t = pconst.tile([128, TT32], f32)
            nc.sync.dma_start(thi_t[:], tok_hi[:])
            tlo_t = pconst.tile([128, TT32], f32)
            nc.sync.dma_start(tlo_t[:], tok_lo[:])
            eid_t = pconst.tile([1, 1], f32)
            nc.sync.dma_start(eid_t[:], eid_d[:])
            eid_b = pconst.tile([128, 1], f32)
            nc.gpsimd.partition_broadcast(eid_b[:], eid_t[:])
            eqe = pconst.tile([128, 8], f32)
            nc.vector.tensor_scalar(eqe[:], iota8_t[:], eid_b[:, 0:1], None, A.is_equal)
            zcol = pconst.tile([128, 1], f32)
            nc.vector.memset(zcol[:], 0.0)

            with tc.tile_pool(name="rpsum", bufs=1,
                              space=bass.MemorySpace.PSUM) as ppr:
                # ---------- router weight int8 quant ----------
                rw_t = prt.tile([8, D], f32)
                nc.sync.dma_start(rw_t[:], rw_d[:])
                rwabs = prt.tile([8, D], f32)
                nc.scalar.activation(rwabs[:], rw_t[:], AF.Abs)
                rcolmax = prt.tile([8, D], f32)
                nc.gpsimd.partition_all_reduce(
                    rcolmax[:], rwabs[:], channels=8,
                    reduce_op=bass_isa.ReduceOp.max)
                ramax = prt.tile([8, 1], f32)
                nc.vector.tensor_reduce(ramax[:], rcolmax[:], axis=X, op=A.max)
                rrec = prt.tile([8, 1], f32)
                nc.vector.reciprocal(rrec[:], ramax[:])
                rs8 = prt.tile([8, 1], f32)
                nc.vector.tensor_scalar_mul(rs8[:], rrec[:], 127.0)
                rq1 = prt.tile([8, D], f32, tag="rwabs")
                nc.vector.tensor_scalar(rq1[:], rw_t[:], rs8[:, 0:1], MAGIC,
                                        A.mult, A.add)
                rsc = prt.tile([8, 1], f32)
                nc.vector.tensor_scalar_mul(rsc[:], ramax[:], 1.0 / 127.0)
                rwq = prt.tile([8, D], f32, tag="rwt")
                nc.vector.tensor_scalar(rwq[:], rq1[:], MAGIC, rsc[:, 0:1],
                                        A.subtract, A.mult)
                rwqT = prt.tile([128, DC, 8], f32)
                for c in range(DC):
                    tp = ppr.tile([128, 8], f32, tag="tp")
                    nc.tensor.transpose(tp[:], rwq[:, c * 128:(c + 1) * 128],
                                        iden8_t[:])
                    nc.scalar.copy(rwqT[:, c, :], tp[:])

                # ---------- router logits ----------
                l_ps = ppr.tile([128, TT32, 8], f32, tag="lps")
                for t in range(TT32):
                    for c in range(DC):
                        xTc = pxt.tile([128, 128], f32, tag="xtc")
                        nc.sync.dma_start(
                            xTc[:], xT_d[c * 128:(c + 1) * 128,
                                         t * 128:(t + 1) * 128])
                        nc.tensor.matmul(
                            l_ps[:, t, :], xTc[:], rwqT[:, c, :],
                            start=(c == 0), stop=(c == DC - 1))
                l_all = prt.tile([128, TT32, 8], f32)
                nc.vector.tensor_copy(l_all[:], l_ps[:])

                # ---------- top-2 ----------
                S3 = (128, TT32, 8)
                m1 = prt.tile([128, TT32], f32)
                nc.vector.tensor_reduce(m1[:], l_all[:], axis=X, op=A.max)
                eq1 = prt.tile([128, TT32, 8], f32)
                nc.vector.tensor_tensor(eq1[:], l_all[:],
                                        m1[:].broadcast_to(S3), A.is_equal)
                lm = prt.tile([128, TT32, 8], f32, tag="lm")
                nc.vector.scalar_tensor_tensor(lm[:], eq1[:], -1e30, l_all[:],
                                               A.mult, A.add)
                m2 = prt.tile([128, TT32], f32)
                nc.vector.tensor_reduce(m2[:], lm[:], axis=X, op=A.max)
                eq2 = prt.tile([128, TT32, 8], f32)
                nc.vector.tensor_tensor(eq2[:], lm[:],
                                        m2[:].broadcast_to(S3), A.is_equal)
                dlt = prt.tile([128, TT32], f32)
                nc.vector.tensor_sub(dlt[:], m1[:], m2[:])
                w1 = prt.tile([128, TT32], f32)
                nc.scalar.activation(w1[:], dlt[:], AF.Sigmoid)
                w2 = prt.tile([128, TT32], f32)
                nc.vector.tensor_scalar(w2[:], w1[:], -1.0, 1.0, A.mult, A.add)

                # ---------- softmax (aux) ----------
                dsub = prt.tile([128, TT32, 8], f32, tag="dsub")
                nc.vector.tensor_tensor(dsub[:], l_all[:],
                                        m1[:].broadcast_to(S3), A.subtract)
                ex = prt.tile([128, TT32, 8], f32, tag="lm")
                nc.scalar.activation(ex[:], dsub[:], AF.Exp)
                exs = prt.tile([128, TT32], f32)
                nc.vector.tensor_reduce(exs[:], ex[:], axis=X, op=A.add)
                exr = prt.tile([128, TT32], f32)
                nc.vector.reciprocal(exr[:], exs[:])
                probs = prt.tile([128, TT32, 8], f32, tag="dsub2")
                nc.vector.tensor_tensor(probs[:], ex[:],
                                        exr[:].broadcast_to(S3), A.mult)
                cnt2 = prt.tile([128, TT32, 8], f32)
                nc.vector.tensor_add(cnt2[:], eq1[:], eq2[:])

                # ---------- capacity cumsum ----------
                cnt_bf = prt.tile([128, TT32, 8], bf16)
                nc.vector.tensor_copy(cnt_bf[:], cnt2[:])
                pref_ps = ppr.tile([128, TT32, 8], f32, tag="prefps")
                tot_ps = ppr.tile([128, TT32, 8], f32, tag="totps")
                for t in range(TT32):
                    nc.tensor.matmul(pref_ps[:, t, :], lt_bf[:], cnt_bf[:, t, :])
                    nc.tensor.matmul(tot_ps[:, t, :], ones_bf[:], cnt_bf[:, t, :])
                tot_sb = prt.tile([128, TT32, 8], f32, tag="dsub")
                nc.vector.tensor_copy(tot_sb[:], tot_ps[:])
                tinc = prt.tile([128, TT32, 8], f32, tag="lm")
                for e in range(8):
                    nc.vector.tensor_tensor_scan(
                        tinc[:, :, e], tot_sb[:, :, e],
                        zcol[:, 0:1].broadcast_to((128, TT32)), 0.0,
                        A.add, A.add)
                pos_all = prt.tile([128, TT32, 8], f32)
                nc.vector.tensor_sub(pos_all[:], tinc[:], tot_sb[:])
                nc.vector.tensor_add(pos_all[:], pos_all[:], pref_ps[:])

                # ---------- our-expert masks/positions ----------
                tmp8 = prt.tile([128, TT32, 8], f32, tag="dsub")
                eqe_bc = eqe[:].broadcast_to((128, 8, TT32)).rearrange(
                    "p e t -> p t e")
                nc.vector.tensor_tensor(tmp8[:], eq1[:], eqe_bc, A.mult)
                mk1 = prt.tile([128, TT32], f32)
                nc.vector.tensor_reduce(mk1[:], tmp8[:], axis=X, op=A.add)
                nc.vector.tensor_tensor(tmp8[:], eq2[:], eqe_bc, A.mult)
                mk2 = prt.tile([128, TT32], f32)
                nc.vector.tensor_reduce(mk2[:], tmp8[:], axis=X, op=A.add)
                nc.vector.tensor_tensor(tmp8[:], pos_all[:], eqe_bc, A.mult)
                pos0 = prt.tile([128, TT32], f32)
                nc.vector.tensor_reduce(pos0[:], tmp8[:], axis=X, op=A.add)
                kle = prt.tile([128, TT32], f32)
                nc.vector.tensor_scalar(kle[:], pos0[:], float(CAP - 1), None,
                                        A.is_le)
                v1 = prt.tile([128, TT32], f32)
                nc.vector.tensor_tensor(v1[:], mk1[:], kle[:], A.mult)
                v2 = prt.tile([128, TT32], f32)
                nc.vector.tensor_tensor(v2[:], mk2[:], kle[:], A.mult)
                vmask = prt.tile([128, TT32], f32)
                nc.vector.tensor_add(vmask[:], v1[:], v2[:])
                cwa = prt.tile([128, TT32], f32)
                nc.vector.tensor_tensor(cwa[:], v1[:], w1[:], A.mult)
                cwb = prt.tile([128, TT32], f32)
                nc.vector.tensor_tensor(cwb[:], v2[:], w2[:], A.mult)
                cwsel = prt.tile([128, TT32], f32)
                nc.vector.tensor_add(cwsel[:], cwa[:], cwb[:])
                posm = prt.tile([128, TT32], f32)
                t1m = prt.tile([128, TT32], f32, tag="cwa")
                nc.vector.tensor_tensor(t1m[:], pos0[:], vmask[:], A.mult)
                t2m = prt.tile([128, TT32], f32, tag="cwb")
                nc.vector.tensor_scalar(t2m[:], vmask[:], 1.0, 1e6,
                                        A.subtract, A.mult)
                nc.vector.tensor_add(posm[:], t1m[:], t2m[:])

                # ---------- values lhsT [128, 32, 5] ----------
                vals = prt.tile([128, TT32, 5], bf16)
                nc.vector.tensor_copy(vals[:, :, 0], thi_t[:])
                nc.vector.tensor_copy(vals[:, :, 1], tlo_t[:])
                nc.vector.tensor_copy(vals[:, :, 2], cwsel[:])
                cwlo = prt.tile([128, TT32], f32, tag="cwa")
                nc.vector.tensor_tensor(cwlo[:], cwsel[:], vals[:, :, 2],
                                        A.subtract)
                nc.vector.tensor_copy(vals[:, :, 3], cwlo[:])
                nc.vector.tensor_copy(vals[:, :, 4], vmask[:])

                # ---------- one-hot scatter -> list [5, 640] ----------
                list_ps0 = ppr.tile([5, 320], f32, tag="listps0")
                list_ps1 = ppr.tile([5, 320], f32, tag="listps1")
                list_ps = [list_ps0, list_ps1]
                for t in range(TT32):
                    oh = prt.tile([128, 640], bf16, tag="oh")
                    nc.vector.tensor_scalar(oh[:], iota640_t[:],
                                            posm[:, t:t + 1], None, A.is_equal)
                    for j in range(2):
                        nc.tensor.matmul(
                            list_ps[j][:], vals[:, t, :],
                            oh[:, j * 320:(j + 1) * 320],
                            start=(t == 0), stop=(t == TT32 - 1))

                # ---------- finalize list ----------
                lrow = psm.tile([5, 640], f32)
                nc.vector.tensor_copy(lrow[:, 0:320], list_ps[0][:])
                nc.vector.tensor_copy(lrow[:, 320:640], list_ps[1][:])
                nc.sync.dma_start(dbg_lrow[:], lrow[:])
                nc.sync.dma_start(dbg_posm[:], posm[:])
                nc.sync.dma_start(dbg_cws[:], cwsel[:])
                nc.sync.dma_start(dbg_lall[:], l_all[:].rearrange("p t e -> p (t e)"))
                valsf = prt.tile([128, TT32, 5], f32, tag="dsub2")
                nc.vector.tensor_copy(valsf[:], vals[:])
                nc.sync.dma_start(dbg_vals[:], valsf[:].rearrange("p t e -> p (t e)"))
                lr_a = psm.tile([1, 640], f32, tag="lra")
                nc.sync.dma_start(lr_a[:], lrow[1:2, :])
                tokf = psm.tile([1, 640], f32, tag="tokf")
                nc.vector.scalar_tensor_tensor(
                    tokf[:], lrow[0:1, :], 64.0, lr_a[:], A.mult, A.add)
                lr_a2 = psm.tile([1, 640], f32, tag="lra")
                nc.sync.dma_start(lr_a2[:], lrow[2:3, :])
                lr_b = psm.tile([1, 640], f32, tag="lrb")
                nc.sync.dma_start(lr_b[:], lrow[3:4, :])
                cwf = psm.tile([1, 640], f32, tag="cwf")
                nc.vector.tensor_tensor(cwf[:], lr_a2[:], lr_b[:], A.add)
                lr_a3 = psm.tile([1, 640], f32, tag="lra")
                nc.sync.dma_start(lr_a3[:], lrow[4:5, :])
                n0f = psm.tile([1, 1], f32)
                nc.vector.tensor_reduce(n0f[:], lr_a3[:], axis=X, op=A.add)
                n0i = psm.tile([1, 1], i32)
                nc.vector.tensor_copy(n0i[:], n0f[:])
                mle = psm.tile([1, 640], f32, tag="lrb")
                nc.vector.tensor_scalar(mle[:], iota640_t[0:1, :],
                                        n0f[0:1, 0:1], None, A.is_lt)
                lfin = psm.tile([1, 640], f32, tag="lra")
                nc.vector.scalar_tensor_tensor(
                    lfin[:], tokf[:], 1.0, mle[:], A.add, A.mult)
                nc.vector.tensor_scalar(lfin[:], lfin[:], 1.0, None, A.subtract)
                li16 = psm.tile([1, 640], i16)
                nc.vector.tensor_copy(li16[:], lfin[:])
                nc.sync.dma_start(li_scratch[:], li16[:])
                nc.sync.dma_start(cw_scratch[:], cwf[:])
                idx16 = psm.tile([128, 40], i16)
                for r in range(8):
                    nc.sync.dma_start(
                        idx16[16 * r:16 * (r + 1), :],
                        li_scratch[:].rearrange("a (c p) -> (a p) c", p=16))
                cw_tl = psm.tile([128, NT], f32)
                nc.sync.dma_start(
                    cw_tl[:], cw_scratch[:].rearrange("a (b p) -> (a p) b", p=128))

                # ---------- aux ----------
                cnt_sum = ppr.tile([1, TT32, 8], f32, tag="auxc")
                prob_sum = ppr.tile([1, TT32, 8], f32, tag="auxp")
                nc.tensor.matmul(cnt_sum[:].rearrange("a b c -> a (b c)"),
                                 onescol_f[:],
                                 cnt2[:].rearrange("p t e -> p (t e)"))
                nc.tensor.matmul(prob_sum[:].rearrange("a b c -> a (b c)"),
                                 onescol_f[:],
                                 probs[:].rearrange("p t e -> p (t e)"))
                cnt_e = psm.tile([1, 8], f32)
                nc.vector.tensor_reduce(
                    cnt_e[:], cnt_sum[:].rearrange("a t e -> a e t"),
                    axis=X, op=A.add)
                prob_e = psm.tile([1, 8], f32)
                nc.vector.tensor_reduce(
                    prob_e[:], prob_sum[:].rearrange("a t e -> a e t"),
                    axis=X, op=A.add)
                fp = psm.tile([1, 8], f32)
                nc.vector.tensor_tensor(fp[:], cnt_e[:], prob_e[:], A.mult)
                auxv = psm.tile([1, 1], f32)
                nc.vector.tensor_reduce(auxv[:], fp[:], axis=X, op=A.add)
                nc.vector.tensor_scalar_mul(auxv[:], auxv[:],
                                            float(NE) / (BT * TOPK * BT))
                nc.sync.dma_start(aux_d[:], auxv[:])

            # ---------- weights + FFN ----------
            with tc.tile_pool(name="fpsum", bufs=2,
                              space=bass.MemorySpace.PSUM) as ppf, \
                 tc.tile_pool(name="fpsum1", bufs=1,
                              space=bass.MemorySpace.PSUM) as ppy, \
                 tc.tile_pool(name="wstream", bufs=2) as pws:

                def quant_weight(w_d, rows, cols, out_tile, out_dt):
                    nchunks = rows // 128
                    NSP = 4 if cols >= 3072 else 2
                    hcol = cols // NSP
                    parts = psm.tile([128, NSP * nchunks], f32, tag="wp" + w_d.name)
                    for ck in range(NSP * nchunks):
                        wc = pws.tile([128, hcol], f32, tag="wst")
                        nc.sync.dma_start(
                            wc[:], w_d[(ck // NSP) * 128:(ck // NSP + 1) * 128,
                                       (ck % NSP) * hcol:(ck % NSP + 1) * hcol])
                        nc.scalar.activation(
                            wc[:], wc[:], AF.Abs,
                            accum_out=parts[:, ck:ck + 1])
                    prow = psm.tile([128, 1], f32, tag="pr" + w_d.name)
                    nc.vector.tensor_reduce(prow[:], parts[:], axis=X, op=A.add)
                    tot2 = ppy.tile([1, 1], f32, tag="wtot")
                    nc.tensor.matmul(tot2[:], onescol_f[:], prow[:])
                    rec = psm.tile([1, 1], f32, tag="rc" + w_d.name)
                    nc.vector.reciprocal(rec[:], tot2[:])
                    rs = psm.tile([1, 1], f32, tag="rs" + w_d.name)
                    nc.vector.tensor_scalar_mul(rs[:], rec[:],
                                                float(rows * cols))
                    rs_b = psm.tile([128, 1], f32, tag="rb" + w_d.name)
                    nc.gpsimd.partition_broadcast(rs_b[:], rs[:])
                    sca = psm.tile([1, 1], f32, tag="sc" + w_d.name)
                    nc.vector.tensor_scalar_mul(sca[:], tot2[:],
                                                1.0 / (rows * cols))
                    s_b = psm.tile([128, 1], f32, tag="sb" + w_d.name)
                    nc.gpsimd.partition_broadcast(s_b[:], sca[:])
                    for ck in range(NSP * nchunks):
                        wc = pws.tile([128, hcol], f32, tag="wst")
                        nc.sync.dma_start(
                            wc[:], w_d[(ck // NSP) * 128:(ck // NSP + 1) * 128,
                                       (ck % NSP) * hcol:(ck % NSP + 1) * hcol])
                        q1 = pffn.tile([128, hcol], f32, tag="wk_a")
                        nc.vector.tensor_scalar(q1[:], wc[:], rs_b[:, 0:1],
                                                MAGIC, A.mult, A.add)
                        q2 = pffn.tile([128, hcol], f32, tag="wk_b")
                        nc.vector.tensor_scalar(q2[:], q1[:], MAGIC - 1.0, 2.0,
                                                A.subtract, A.min)
                        nc.vector.tensor_scalar(
                            out_tile[:, ck // NSP,
                                     (ck % NSP) * hcol:(ck % NSP + 1) * hcol],
                            q2[:], 0.0, None, A.max)
                    return s_b

                gq = pwq.tile([128, DC, FFN], fp8, tag="gq")
                uq = pwq.tile([128, DC, FFN], fp8, tag="uq")
                dq = pwq.tile([128, KC, D], bf16, tag="dq")
                sg_b = quant_weight(gwT_d, D, FFN, gq, fp8)
                su_b = quant_weight(uwT_d, D, FFN, uq, fp8)
                sd_b = quant_weight(dwT_d, FFN, D, dq, bf16)

                # ---------- gather + int4 quant ----------
                n0_reg = nc.gpsimd.alloc_register("n0reg")
                xg = phb.tile([128, NT, D], f32, tag="xgy")
                dgsem = nc.alloc_semaphore("dgsem")
                with tc.tile_critical():
                    nc.gpsimd.load(n0_reg, n0i[0:1, 0:1])
                    nc.gpsimd.dma_gather(
                        xg[:], x_d[:, :], idx16[:], 640, n0_reg, D
                    ).then_inc(dgsem, 16)
                    nc.gpsimd.wait_ge(dgsem, 16)
                amax5 = psm.tile([128, NT], f32)
                nc.vector.tensor_reduce(amax5[:], xg[:], axis=X, op=A.max,
                                        apply_absolute_value=True)
                r5 = psm.tile([128, NT], f32)
                nc.vector.reciprocal(r5[:], amax5[:])
                s7 = psm.tile([128, NT], f32)
                nc.vector.tensor_scalar_mul(s7[:], r5[:], 7.0)
                xq = pffn.tile([128, NT, D], bf16, tag="xq")
                sumxq = psm.tile([128, NT], f32)
                for t in range(NT):
                    xv1 = pffn.tile([128, D], f32, tag="xv1")
                    nc.vector.tensor_scalar(xv1[:], xg[:, t, :], s7[:, t:t + 1],
                                            MAGIC, A.mult, A.add)
                    nc.vector.tensor_scalar(xq[:, t, :], xv1[:], MAGIC, 0.0,
                                            A.subtract, A.add,
                                            accum_out=sumxq[:, t:t + 1])
                xqT = pffn.tile([128, NT, DC, 128], fp8, tag="xqT")
                for t in range(NT):
                    for c in range(DC):
                        xtt = pffn.tile([128, 128], bf16, tag="xtt")
                        nc.sync.dma_start_transpose(
                            xtt[:], xq[:, t, c * 128:(c + 1) * 128])
                        nc.vector.tensor_copy(xqT[:, t, c, :], xtt[:])

                gsc = psm.tile([128, NT], f32)
                nc.vector.tensor_tensor(gsc[:], amax5[:],
                                        sg_b[:, 0:1].broadcast_to((128, NT)),
                                        A.mult)
                nc.vector.tensor_scalar_mul(gsc[:], gsc[:], 1.0 / 7.0)
                gbias = psm.tile([128, NT], f32)
                nc.vector.tensor_tensor(gbias[:], sumxq[:], gsc[:], A.mult)
                nc.vector.tensor_scalar_mul(gbias[:], gbias[:], -1.0)
                usc = psm.tile([128, NT], f32)
                nc.vector.tensor_tensor(usc[:], amax5[:],
                                        su_b[:, 0:1].broadcast_to((128, NT)),
                                        A.mult)
                nc.vector.tensor_scalar_mul(usc[:], usc[:], 1.0 / 7.0)

                yout = phb.tile([128, NT, D], f32, tag="xgy")
                act_fn = AF.Sigmoid if sim_sigmoid else AF.Silu

                for t in range(NT):
                    htil = phb.tile([128, FFN], f32, tag="htil")
                    for fc in range(FC):
                        g_ps = ppf.tile([128, 512], f32, tag="gps")
                        u_ps = ppf.tile([128, 512], f32, tag="ups")
                        for c in range(DC):
                            nc.tensor.matmul(
                                g_ps[:], xqT[:, t, c, :],
                                gq[:, c, fc * 512:(fc + 1) * 512],
                                start=(c == 0), stop=(c == DC - 1))
                        for c in range(DC):
                            nc.tensor.matmul(
                                u_ps[:], xqT[:, t, c, :],
                                uq[:, c, fc * 512:(fc + 1) * 512],
                                start=(c == 0), stop=(c == DC - 1))
                        sgl = pffn.tile([128, 512], f32, tag="sgl")
                        nc.scalar.activation(
                            sgl[:], g_ps[:], act_fn,
                            bias=gbias[:, t:t + 1], scale=gsc[:, t:t + 1])
                        ucc = pffn.tile([128, 512], f32, tag="ucc")
                        nc.vector.tensor_scalar(
                            ucc[:], u_ps[:], sumxq[:, t:t + 1], None,
                            A.subtract)
                        nc.vector.tensor_tensor(
                            htil[:, fc * 512:(fc + 1) * 512], sgl[:], ucc[:],
                            A.mult)
                    amaxh = psm.tile([128, 1], f32, tag="amaxh")
                    nc.vector.tensor_reduce(amaxh[:], htil[:], axis=X, op=A.max,
                                            apply_absolute_value=True)
                    lo = psm.tile([128, 1], f32, tag="lo")
                    nc.vector.memset(lo[:], 0.0)
                    hi = psm.tile([128, 1], f32, tag="hi")
                    nc.vector.tensor_copy(hi[:], amaxh[:])
                    mid = psm.tile([128, 1], f32, tag="mid")
                    nmid = psm.tile([128, 1], f32, tag="nmid")
                    cnta = psm.tile([128, 1], f32, tag="cnta")
                    sgn = psm.tile([128, 1], f32, tag="sgn")
                    c2 = psm.tile([128, 1], f32, tag="c2")
                    junk = pffn.tile([128, FFN], bf16, tag="hq")
                    junka = pffn.tile([128, FFN], bf16, tag="msk")
                    # count(h >= mid) on DVE; count(h <= -mid) via ACT
                    # sign-accumulate: sum sign(-h - mid) = #lt - #gt, so
                    # cntb ~= (FFN + sum)/2 (ties measure-zero). The keep
                    # test cnta + cntb >= KTH becomes
                    # cnta + sum/2 >= KTH - FFN/2.
                    for it in range(SEARCH_ITERS):
                        nc.vector.tensor_add(mid[:], lo[:], hi[:])
                        nc.vector.tensor_scalar_mul(mid[:], mid[:], 0.5)
                        nc.vector.tensor_scalar_mul(nmid[:], mid[:], -1.0)
                        nc.vector.tensor_scalar(
                            junk[:], htil[:], mid[:, 0:1], 0.0,
                            A.is_ge, A.add, accum_out=cnta[:])
                        nc.scalar.activation(
                            junka[:], htil[:], AF.Sign,
                            bias=nmid[:, 0:1], scale=-1.0, accum_out=sgn[:])
                        nc.vector.scalar_tensor_tensor(
                            c2[:], sgn[:], 0.5, cnta[:], A.mult, A.add)
                        ge = psm.tile([128, 1], mybir.dt.uint8, tag="ge")
                        nc.vector.tensor_scalar(ge[:], c2[:],
                                                float(KTH - FFN // 2),
                                                None, A.is_ge)
                        nc.vector.copy_predicated(lo[:], ge[:], mid[:])
                        gei = psm.tile([128, 1], mybir.dt.uint8, tag="gei")
                        nc.vector.tensor_scalar(gei[:], c2[:],
                                                float(KTH - FFN // 2),
                                                None, A.is_lt)
                        nc.vector.copy_predicated(hi[:], gei[:], mid[:])
                    s8r = psm.tile([128, 1], f32, tag="s8r")
                    nc.vector.reciprocal(s8r[:], amaxh[:])
                    s8 = psm.tile([128, 1], f32, tag="s8")
                    nc.vector.tensor_scalar_mul(s8[:], s8r[:], 127.0)
                    nlo = psm.tile([128, 1], f32, tag="nmid")
                    nc.vector.tensor_scalar_mul(nlo[:], lo[:], -1.0)
                    mska = pffn.tile([128, FFN], bf16, tag="hqT")
                    nc.vector.tensor_scalar(mska[:], htil[:], lo[:, 0:1], None,
                                            A.is_ge)
                    msk = pffn.tile([128, FFN], bf16, tag="msk")
                    nc.vector.tensor_scalar(msk[:], htil[:], nlo[:, 0:1], 1.0,
                                            A.is_le, A.mult)
                    nc.vector.tensor_add(msk[:], msk[:], mska[:])
                    nc.vector.tensor_scalar(htil[:], htil[:], s8[:, 0:1], MAGIC,
                                            A.mult, A.add)
                    hq = pffn.tile([128, FFN], bf16, tag="hq")
                    sumq = psm.tile([128, 1], f32, tag="sumq")
                    nc.vector.scalar_tensor_tensor(
                        hq[:], htil[:], MAGIC, msk[:], A.subtract, A.mult,
                        accum_out=sumq[:])
                    hqT = pffn.tile([128, KC, 128], bf16, tag="hqT")
                    for c in range(KC):
                        nc.sync.dma_start_transpose(
                            hqT[:, c, :], hq[:, c * 128:(c + 1) * 128])
                    y_ps0 = ppy.tile([128, 384], f32, tag="yps0")
                    y_ps1 = ppy.tile([128, 384], f32, tag="yps1")
                    y_ps = [y_ps0, y_ps1]
                    for j in range(2):
                        for c in range(KC):
                            nc.tensor.matmul(
                                y_ps[j][:], hqT[:, c, :],
                                dq[:, c, j * 384:(j + 1) * 384],
                                start=(c == 0), stop=(c == KC - 1))
                    fsc = psm.tile([128, 1], f32, tag="fsc")
                    nc.vector.tensor_tensor(fsc[:], amaxh[:], usc[:, t:t + 1],
                                            A.mult)
                    nc.vector.tensor_tensor(fsc[:], fsc[:], sd_b[:, 0:1],
                                            A.mult)
                    nc.vector.tensor_tensor(fsc[:], fsc[:], cw_tl[:, t:t + 1],
                                            A.mult)
                    nc.vector.tensor_scalar_mul(fsc[:], fsc[:], 1.0 / 127.0)
                    for j in range(2):
                        nc.vector.tensor_scalar(
                            yout[:, t, j * 384:(j + 1) * 384], y_ps[j][:],
                            sumq[:, 0:1], fsc[:, 0:1], A.subtract, A.mult)

                scsem = nc.alloc_semaphore("scsem")
                with tc.tile_critical():
                    nc.gpsimd.dma_scatter_add(
                        out_d[:, :], yout[:], idx16[:], 640, n0_reg, D
                    ).then_inc(scsem, 16)
                    nc.gpsimd.wait_ge(scsem, 16)

    nc.compile()
    return nc


_NC_CACHE = {}


def _get_nc(sim_sigmoid=False):
    key = bool(sim_sigmoid)
    if key not in _NC_CACHE:
        _NC_CACHE[key] = build(sim_sigmoid=key)
    return _NC_CACHE[key]


def make_in_maps(x, gate_w, up_w, down_w, router_w):
    x2 = np.ascontiguousarray(np.asarray(x).reshape(BT, D).astype(np.float32))
    xT = np.ascontiguousarray(x2.T)
    rw = np.ascontiguousarray(np.asarray(router_w).astype(np.float32))
    in_maps = []
    for e in range(NE):
        in_maps.append({
            "x": x2,
            "xT": xT,
            "gwT": np.ascontiguousarray(np.asarray(gate_w[e]).T.astype(np.float32)),
            "uwT": np.ascontiguousarray(np.asarray(up_w[e]).T.astype(np.float32)),
            "dwT": np.ascontiguousarray(np.asarray(down_w[e]).T.astype(np.float32)),
            "rw": rw,
            "eid": np.array([[float(e)]], dtype=np.float32),
        })
    return in_maps


def kernel(x, gate_w, up_w, down_w, router_w):
    from concourse.bass_utils import run_bass_kernel_spmd

    nc = _get_nc(sim_sigmoid=False)
    in_maps = make_in_maps(x, gate_w, up_w, down_w, router_w)
    res = run_bass_kernel_spmd(nc, in_maps, core_ids=list(range(NE)))
    out = np.zeros((BT, D), np.float32)
    for e in range(NE):
        out += res.results[e]["out"]
    aux = np.float32(res.results[0]["aux"][0, 0])
    return out.reshape(B, T, D), aux


# revision 23
# speedup vs baseline: 1.1643x; 1.0638x over previous
"""BitMoE FFN (8 experts, top-2, capacity 640) on 8 TRN2 NeuronCores.

Expert-parallel: core i owns expert i (ternary weights quantized on device),
router replicated. Per core: f32 router logits on PE -> top-2 + capacity
(exclusive cumsum in slot order) -> compact token list via one-hot matmuls ->
dma_gather of x rows -> exact-integer fp8/bf16 matmuls for gate/up ->
silu*u -> per-token top-55% |h| threshold (binary search with fused
abs+compare+count) -> masked int8 quant -> ternary down matmul -> combine
scale -> dma_scatter_add into this core's partial output. Host sums the 8
partial outputs.
"""
import os
import sys
import numpy as np

sys.path.insert(0, "/opt/trn_rl_repo")

B, T, D = 4, 1024, 768
BT = B * T
FFN = 3072
NE = 8
TOPK = 2
CAP = 640
KTH = 1690
NT = CAP // 128       # 5
DC = D // 128         # 6
FC = FFN // 512       # 6
KC = FFN // 128       # 24
TT32 = BT // 128      # 32
MAGIC = 12582912.0    # 2**23 + 2**22
SEARCH_ITERS = int(os.environ.get("KB_SEARCH_ITERS", "13"))


def build(sim_sigmoid=False):
    import concourse.bacc as bacc
    import concourse.mybir as mybir
    import concourse.bass_isa as bass_isa
    from concourse import tile
    from concourse import bass

    f32 = mybir.dt.float32
    bf16 = mybir.dt.bfloat16
    fp8 = mybir.dt.float8e4
    i16 = mybir.dt.int16
    i32 = mybir.dt.int32
    A = mybir.AluOpType
    AF = mybir.ActivationFunctionType
    X = mybir.AxisListType.X

    nc = bacc.Bacc(trn_type="TRN2")

    x_d = nc.dram_tensor("x", [BT, D], f32, kind="ExternalInput")
    xT_d = nc.dram_tensor("xT", [D, BT], f32, kind="ExternalInput")
    gwT_d = nc.dram_tensor("gwT", [D, FFN], f32, kind="ExternalInput")
    uwT_d = nc.dram_tensor("uwT", [D, FFN], f32, kind="ExternalInput")
    dwT_d = nc.dram_tensor("dwT", [FFN, D], f32, kind="ExternalInput")
    rw_d = nc.dram_tensor("rw", [NE, D], f32, kind="ExternalInput")
    eid_d = nc.dram_tensor("eid", [1, 1], f32, kind="ExternalInput")
    out_d = nc.dram_tensor("out", [BT, D], f32, kind="ExternalOutput")
    aux_d = nc.dram_tensor("aux", [1, 1], f32, kind="ExternalOutput")
    li_scratch = nc.dram_tensor("li_scr", [1, 640], i16, kind="Internal")
    dbg_lrow = nc.dram_tensor("dbg_lrow", [5, 640], f32, kind="Internal")
    dbg_posm = nc.dram_tensor("dbg_posm", [128, TT32], f32, kind="Internal")
    dbg_cws = nc.dram_tensor("dbg_cws", [128, TT32], f32, kind="Internal")
    dbg_lall = nc.dram_tensor("dbg_lall", [128, TT32 * 8], f32, kind="Internal")
    dbg_vals = nc.dram_tensor("dbg_vals", [128, TT32 * 5], f32, kind="Internal")
    cw_scratch = nc.dram_tensor("cw_scr", [1, 640], f32, kind="Internal")

    iden8 = nc.inline_tensor(np.eye(8, dtype=np.float32), name="iden8")
    lt_np = np.fromfunction(lambda k, m: (k < m), (128, 128)).astype(np.float32)
    ltc = nc.inline_tensor(lt_np, name="ltc")
    ones128 = nc.inline_tensor(np.ones((128, 128), np.float32), name="ones128")
    onescol = nc.inline_tensor(np.ones((128, 1), np.float32), name="onescol")
    iota8 = nc.inline_tensor(
        np.tile(np.arange(8, dtype=np.float32)[None, :], (128, 1)), name="iota8")
    iota640 = nc.inline_tensor(
        np.tile(np.arange(640, dtype=np.float32)[None, :], (128, 1)), name="iota640")
    tok_hi = nc.inline_tensor(
        np.fromfunction(lambda p, t: np.floor((t * 128 + p) / 64), (128, TT32)
                        ).astype(np.float32), name="tok_hi")
    tok_lo = nc.inline_tensor(
        np.fromfunction(lambda p, t: (t * 128 + p) % 64, (128, TT32)
                        ).astype(np.float32), name="tok_lo")

    with tile.TileContext(nc) as tc:
        with (
            tc.tile_pool(name="consts", bufs=1) as pconst,
            tc.tile_pool(name="wq", bufs=1) as pwq,
            tc.tile_pool(name="router", bufs=1) as prt,
            tc.tile_pool(name="xts", bufs=4) as pxt,
            tc.tile_pool(name="wrk", bufs=1) as pffn,
            tc.tile_pool(name="hbuf", bufs=1) as phb,
            tc.tile_pool(name="small", bufs=1) as psm,
        ):
            # ---------- constants ----------
            iden8_t = pconst.tile([8, 8], f32)
            nc.sync.dma_start(iden8_t[:], iden8[:])
            ltf = pconst.tile([128, 128], f32, tag="cf128")
            nc.sync.dma_start(ltf[:], ltc[:])
            lt_bf = pconst.tile([128, 128], bf16)
            nc.vector.tensor_copy(lt_bf[:], ltf[:])
            onesf2 = pconst.tile([128, 128], f32, tag="cf128b")
            nc.sync.dma_start(onesf2[:], ones128[:])
            ones_bf = pconst.tile([128, 128], bf16)
            nc.vector.tensor_copy(ones_bf[:], onesf2[:])
            onescol_f = pconst.tile([128, 1], f32)
            nc.sync.dma_start(onescol_f[:], onescol[:])
            iota8_t = pconst.tile([128, 8], f32)
            nc.sync.dma_start(iota8_t[:], iota8[:])
            iota640_t = pconst.tile([128, 640], f32)
            nc.sync.dma_start(iota640_t[:], iota640[:])
            thi_t = pconst.tile([128, TT32], f32)
            nc.sync.dma_start(thi_t[:], tok_hi[:])
            tlo_t = pconst.tile([128, TT32], f32)
            nc.sync.dma_start(tlo_t[:], tok_lo[:])
            eid_t = pconst.tile([1, 1], f32)
            nc.sync.dma_start(eid_t[:], eid_d[:])
            eid_b = pconst.tile([128, 1], f32)
            nc.gpsimd.partition_broadcast(eid_b[:], eid_t[:])
            eqe = pconst.tile([128, 8], f32)
            nc.vector.tensor_scalar(eqe[:], iota8_t[:], eid_b[:, 0:1], None, A.is_equal)
            zcol = pconst.tile([128, 1], f32)
            nc.vector.memset(zcol[:], 0.0)

            with tc.tile_pool(name="rpsum", bufs=1,
                              space=bass.MemorySpace.PSUM) as ppr:
                # ---------- router weight int8 quant ----------
                rw_t = prt.tile([8, D], f32)
                nc.sync.dma_start(rw_t[:], rw_d[:])
                rwabs = prt.tile([8, D], f32)
                nc.scalar.activation(rwabs[:], rw_t[:], AF.Abs)
                rcolmax = prt.tile([8, D], f32)
                nc.gpsimd.partition_all_reduce(
                    rcolmax[:], rwabs[:], channels=8,
                    reduce_op=bass_isa.ReduceOp.max)
                ramax = prt.tile([8, 1], f32)
                nc.vector.tensor_reduce(ramax[:], rcolmax[:], axis=X, op=A.max)
                rrec = prt.tile([8, 1], f32)
                nc.vector.reciprocal(rrec[:], ramax[:])
                rs8 = prt.tile([8, 1], f32)
                nc.vector.tensor_scalar_mul(rs8[:], rrec[:], 127.0)
                rq1 = prt.tile([8, D], f32, tag="rwabs")
                nc.vector.tensor_scalar(rq1[:], rw_t[:], rs8[:, 0:1], MAGIC,
                                        A.mult, A.add)
                rsc = prt.tile([8, 1], f32)
                nc.vector.tensor_scalar_mul(rsc[:], ramax[:], 1.0 / 127.0)
                rwq = prt.tile([8, D], f32, tag="rwt")
                nc.vector.tensor_scalar(rwq[:], rq1[:], MAGIC, rsc[:, 0:1],
                                        A.subtract, A.mult)
                rwqT = prt.tile([128, DC, 8], f32)
                for c in range(DC):
                    tp = ppr.tile([128, 8], f32, tag="tp")
                    nc.tensor.transpose(tp[:], rwq[:, c * 128:(c + 1) * 128],
                                        iden8_t[:])
                    nc.scalar.copy(rwqT[:, c, :], tp[:])

                # ---------- router logits ----------
                l_ps = ppr.tile([128, TT32, 8], f32, tag="lps")
                for t in range(TT32):
                    for c in range(DC):
                        xTc = pxt.tile([128, 128], f32, tag="xtc")
                        nc.sync.dma_start(
                            xTc[:], xT_d[c * 128:(c + 1) * 128,
                                         t * 128:(t + 1) * 128])
                        nc.tensor.matmul(
                            l_ps[:, t, :], xTc[:], rwqT[:, c, :],
                            start=(c == 0), stop=(c == DC - 1))
                l_all = prt.tile([128, TT32, 8], f32)
                nc.vector.tensor_copy(l_all[:], l_ps[:])

                # ---------- top-2 ----------
                S3 = (128, TT32, 8)
                m1 = prt.tile([128, TT32], f32)
                nc.vector.tensor_reduce(m1[:], l_all[:], axis=X, op=A.max)
                eq1 = prt.tile([128, TT32, 8], f32)
                nc.vector.tensor_tensor(eq1[:], l_all[:],
                                        m1[:].broadcast_to(S3), A.is_equal)
                lm = prt.tile([128, TT32, 8], f32, tag="lm")
                nc.vector.scalar_tensor_tensor(lm[:], eq1[:], -1e30, l_all[:],
                                               A.mult, A.add)
                m2 = prt.tile([128, TT32], f32)
                nc.vector.tensor_reduce(m2[:], lm[:], axis=X, op=A.max)
                eq2 = prt.tile([128, TT32, 8], f32)
                nc.vector.tensor_tensor(eq2[:], lm[:],
                                        m2[:].broadcast_to(S3), A.is_equal)
                dlt = prt.tile([128, TT32], f32)
                nc.vector.tensor_sub(dlt[:], m1[:], m2[:])
                w1 = prt.tile([128, TT32], f32)
                nc.scalar.activation(w1[:], dlt[:], AF.Sigmoid)
                w2 = prt.tile([128, TT32], f32)
                nc.vector.tensor_scalar(w2[:], w1[:], -1.0, 1.0, A.mult, A.add)

                # ---------- softmax (aux) ----------
                dsub = prt.tile([128, TT32, 8], f32, tag="dsub")
                nc.vector.tensor_tensor(dsub[:], l_all[:],
                                        m1[:].broadcast_to(S3), A.subtract)
                ex = prt.tile([128, TT32, 8], f32, tag="lm")
                nc.scalar.activation(ex[:], dsub[:], AF.Exp)
                exs = prt.tile([128, TT32], f32)
                nc.vector.tensor_reduce(exs[:], ex[:], axis=X, op=A.add)
                exr = prt.tile([128, TT32], f32)
                nc.vector.reciprocal(exr[:], exs[:])
                probs = prt.tile([128, TT32, 8], f32, tag="dsub2")
                nc.vector.tensor_tensor(probs[:], ex[:],
                                        exr[:].broadcast_to(S3), A.mult)
                cnt2 = prt.tile([128, TT32, 8], f32)
                nc.vector.tensor_add(cnt2[:], eq1[:], eq2[:])

                # ---------- capacity cumsum ----------
                cnt_bf = prt.tile([128, TT32, 8], bf16)
                nc.vector.tensor_copy(cnt_bf[:], cnt2[:])
                pref_ps = ppr.tile([128, TT32, 8], f32, tag="prefps")
                tot_ps = ppr.tile([128, TT32, 8], f32, tag="totps")
                for t in range(TT32):
                    nc.tensor.matmul(pref_ps[:, t, :], lt_bf[:], cnt_bf[:, t, :])
                    nc.tensor.matmul(tot_ps[:, t, :], ones_bf[:], cnt_bf[:, t, :])
                tot_sb = prt.tile([128, TT32, 8], f32, tag="dsub")
                nc.vector.tensor_copy(tot_sb[:], tot_ps[:])
                tinc = prt.tile([128, TT32, 8], f32, tag="lm")
                for e in range(8):
                    nc.vector.tensor_tensor_scan(
                        tinc[:, :, e], tot_sb[:, :, e],
                        zcol[:, 0:1].broadcast_to((128, TT32)), 0.0,
                        A.add, A.add)
                pos_all = prt.tile([128, TT32, 8], f32)
                nc.vector.tensor_sub(pos_all[:], tinc[:], tot_sb[:])
                nc.vector.tensor_add(pos_all[:], pos_all[:], pref_ps[:])

                # ---------- our-expert masks/positions ----------
                tmp8 = prt.tile([128, TT32, 8], f32, tag="dsub")
                eqe_bc = eqe[:].broadcast_to((128, 8, TT32)).rearrange(
                    "p e t -> p t e")
                nc.vector.tensor_tensor(tmp8[:], eq1[:], eqe_bc, A.mult)
                mk1 = prt.tile([128, TT32], f32)
                nc.vector.tensor_reduce(mk1[:], tmp8[:], axis=X, op=A.add)
                nc.vector.tensor_tensor(tmp8[:], eq2[:], eqe_bc, A.mult)
                mk2 = prt.tile([128, TT32], f32)
                nc.vector.tensor_reduce(mk2[:], tmp8[:], axis=X, op=A.add)
                nc.vector.tensor_tensor(tmp8[:], pos_all[:], eqe_bc, A.mult)
                pos0 = prt.tile([128, TT32], f32)
                nc.vector.tensor_reduce(pos0[:], tmp8[:], axis=X, op=A.add)
                kle = prt.tile([128, TT32], f32)
                nc.vector.tensor_scalar(kle[:], pos0[:], float(CAP - 1), None,
                                        A.is_le)
                v1 = prt.tile([128, TT32], f32)
                nc.vector.tensor_tensor(v1[:], mk1[:], kle[:], A.mult)
                v2 = prt.tile([128, TT32], f32)
                nc.vector.tensor_tensor(v2[:], mk2[:], kle[:], A.mult)
                vmask = prt.tile([128, TT32], f32)
                nc.vector.tensor_add(vmask[:], v1[:], v2[:])
                cwa = prt.tile([128, TT32], f32)
                nc.vector.tensor_tensor(cwa[:], v1[:], w1[:], A.mult)
                cwb = prt.tile([128, TT32], f32)
                nc.vector.tensor_tensor(cwb[:], v2[:], w2[:], A.mult)
                cwsel = prt.tile([128, TT32], f32)
                nc.vector.tensor_add(cwsel[:], cwa[:], cwb[:])
                posm = prt.tile([128, TT32], f32)
                t1m = prt.tile([128, TT32], f32, tag="cwa")
                nc.vector.tensor_tensor(t1m[:], pos0[:], vmask[:], A.mult)
                t2m = prt.tile([128, TT32], f32, tag="cwb")
                nc.vector.tensor_scalar(t2m[:], vmask[:], 1.0, 1e6,
                                        A.subtract, A.mult)
                nc.vector.tensor_add(posm[:], t1m[:], t2m[:])

                # ---------- values lhsT [128, 32, 5] ----------
                vals = prt.tile([128, TT32, 5], bf16)
                nc.vector.tensor_copy(vals[:, :, 0], thi_t[:])
                nc.vector.tensor_copy(vals[:, :, 1], tlo_t[:])
                nc.vector.tensor_copy(vals[:, :, 2], cwsel[:])
                cwlo = prt.tile([128, TT32], f32, tag="cwa")
                nc.vector.tensor_tensor(cwlo[:], cwsel[:], vals[:, :, 2],
                                        A.subtract)
                nc.vector.tensor_copy(vals[:, :, 3], cwlo[:])
                nc.vector.tensor_copy(vals[:, :, 4], vmask[:])

                # ---------- one-hot scatter -> list [5, 640] ----------
                list_ps0 = ppr.tile([5, 320], f32, tag="listps0")
                list_ps1 = ppr.tile([5, 320], f32, tag="listps1")
                list_ps = [list_ps0, list_ps1]
                for t in range(TT32):
                    oh = prt.tile([128, 640], bf16, tag="oh")
                    nc.vector.tensor_scalar(oh[:], iota640_t[:],
                                            posm[:, t:t + 1], None, A.is_equal)
                    for j in range(2):
                        nc.tensor.matmul(
                            list_ps[j][:], vals[:, t, :],
                            oh[:, j * 320:(j + 1) * 320],
                            start=(t == 0), stop=(t == TT32 - 1))

                # ---------- finalize list ----------
                lrow = psm.tile([5, 640], f32)
                nc.vector.tensor_copy(lrow[:, 0:320], list_ps[0][:])
                nc.vector.tensor_copy(lrow[:, 320:640], list_ps[1][:])
                nc.sync.dma_start(dbg_lrow[:], lrow[:])
                nc.sync.dma_start(dbg_posm[:], posm[:])
                nc.sync.dma_start(dbg_cws[:], cwsel[:])
                nc.sync.dma_start(dbg_lall[:], l_all[:].rearrange("p t e -> p (t e)"))
                valsf = prt.tile([128, TT32, 5], f32, tag="dsub2")
                nc.vector.tensor_copy(valsf[:], vals[:])
                nc.sync.dma_start(dbg_vals[:], valsf[:].rearrange("p t e -> p (t e)"))
                lr_a = psm.tile([1, 640], f32, tag="lra")
                nc.sync.dma_start(lr_a[:], lrow[1:2, :])
                tokf = psm.tile([1, 640], f32, tag="tokf")
                nc.vector.scalar_tensor_tensor(
                    tokf[:], lrow[0:1, :], 64.0, lr_a[:], A.mult, A.add)
                lr_a2 = psm.tile([1, 640], f32, tag="lra")
                nc.sync.dma_start(lr_a2[:], lrow[2:3, :])
                lr_b = psm.tile([1, 640], f32, tag="lrb")
                nc.sync.dma_start(lr_b[:], lrow[3:4, :])
                cwf = psm.tile([1, 640], f32, tag="cwf")
                nc.vector.tensor_tensor(cwf[:], lr_a2[:], lr_b[:], A.add)
                lr_a3 = psm.tile([1, 640], f32, tag="lra")
                nc.sync.dma_start(lr_a3[:], lrow[4:5, :])
                n0f = psm.tile([1, 1], f32)
                nc.vector.tensor_reduce(n0f[:], lr_a3[:], axis=X, op=A.add)
                n0i = psm.tile([1, 1], i32)
                nc.vector.tensor_copy(n0i[:], n0f[:])
                mle = psm.tile([1, 640], f32, tag="lrb")
                nc.vector.tensor_scalar(mle[:], iota640_t[0:1, :],
                                        n0f[0:1, 0:1], None, A.is_lt)
                lfin = psm.tile([1, 640], f32, tag="lra")
                nc.vector.scalar_tensor_tensor(
                    lfin[:], tokf[:], 1.0, mle[:], A.add, A.mult)
                nc.vector.tensor_scalar(lfin[:], lfin[:], 1.0, None, A.subtract)
                li16 = psm.tile([1, 640], i16)
                nc.vector.tensor_copy(li16[:], lfin[:])
                nc.sync.dma_start(li_scratch[:], li16[:])
                nc.sync.dma_start(cw_scratch[:], cwf[:])
                idx16 = psm.tile([128, 40], i16)
                for r in range(8):
                    nc.sync.dma_start(
                        idx16[16 * r:16 * (r + 1), :],
                        li_scratch[:].rearrange("a (c p) -> (a p) c", p=16))
                cw_tl = psm.tile([128, NT], f32)
                nc.sync.dma_start(
                    cw_tl[:], cw_scratch[:].rearrange("a (b p) -> (a p) b", p=128))

                # ---------- aux ----------
                cnt_sum = ppr.tile([1, TT32, 8], f32, tag="auxc")
                prob_sum = ppr.tile([1, TT32, 8], f32, tag="auxp")
                nc.tensor.matmul(cnt_sum[:].rearrange("a b c -> a (b c)"),
                                 onescol_f[:],
                                 cnt2[:].rearrange("p t e -> p (t e)"))
                nc.tensor.matmul(prob_sum[:].rearrange("a b c -> a (b c)"),
                                 onescol_f[:],
                                 probs[:].rearrange("p t e -> p (t e)"))
                cnt_e = psm.tile([1, 8], f32)
                nc.vector.tensor_reduce(
                    cnt_e[:], cnt_sum[:].rearrange("a t e -> a e t"),
                    axis=X, op=A.add)
                prob_e = psm.tile([1, 8], f32)
                nc.vector.tensor_reduce(
                    prob_e[:], prob_sum[:].rearrange("a t e -> a e t"),
                    axis=X, op=A.add)
                fp = psm.tile([1, 8], f32)
                nc.vector.tensor_tensor(fp[:], cnt_e[:], prob_e[:], A.mult)
                auxv = psm.tile([1, 1], f32)
                nc.vector.tensor_reduce(auxv[:], fp[:], axis=X, op=A.add)
                nc.vector.tensor_scalar_mul(auxv[:], auxv[:],
                                            float(NE) / (BT * TOPK * BT))
                nc.sync.dma_start(aux_d[:], auxv[:])

            # ---------- weights + FFN ----------
            with tc.tile_pool(name="fpsum", bufs=2,
                              space=bass.MemorySpace.PSUM) as ppf, \
                 tc.tile_pool(name="fpsum1", bufs=1,
                              space=bass.MemorySpace.PSUM) as ppy, \
                 tc.tile_pool(name="wstream", bufs=2) as pws:

                def quant_weight(w_d, rows, cols, out_tile, out_dt):
                    nchunks = rows // 128
                    NSP = 4 if cols >= 3072 else 2
                    hcol = cols // NSP
                    parts = psm.tile([128, NSP * nchunks], f32, tag="wp" + w_d.name)
                    for ck in range(NSP * nchunks):
                        wc = pws.tile([128, hcol], f32, tag="wst")
                        nc.sync.dma_start(
                            wc[:], w_d[(ck // NSP) * 128:(ck // NSP + 1) * 128,
                                       (ck % NSP) * hcol:(ck % NSP + 1) * hcol])
                        nc.scalar.activation(
                            wc[:], wc[:], AF.Abs,
                            accum_out=parts[:, ck:ck + 1])
                    prow = psm.tile([128, 1], f32, tag="pr" + w_d.name)
                    nc.vector.tensor_reduce(prow[:], parts[:], axis=X, op=A.add)
                    tot2 = ppy.tile([1, 1], f32, tag="wtot")
                    nc.tensor.matmul(tot2[:], onescol_f[:], prow[:])
                    rec = psm.tile([1, 1], f32, tag="rc" + w_d.name)
                    nc.vector.reciprocal(rec[:], tot2[:])
                    rs = psm.tile([1, 1], f32, tag="rs" + w_d.name)
                    nc.vector.tensor_scalar_mul(rs[:], rec[:],
                                                float(rows * cols))
                    rs_b = psm.tile([128, 1], f32, tag="rb" + w_d.name)
                    nc.gpsimd.partition_broadcast(rs_b[:], rs[:])
                    sca = psm.tile([1, 1], f32, tag="sc" + w_d.name)
                    nc.vector.tensor_scalar_mul(sca[:], tot2[:],
                                                1.0 / (rows * cols))
                    s_b = psm.tile([128, 1], f32, tag="sb" + w_d.name)
                    nc.gpsimd.partition_broadcast(s_b[:], sca[:])
                    for ck in range(NSP * nchunks):
                        wc = pws.tile([128, hcol], f32, tag="wst")
                        nc.sync.dma_start(
                            wc[:], w_d[(ck // NSP) * 128:(ck // NSP + 1) * 128,
                                       (ck % NSP) * hcol:(ck % NSP + 1) * hcol])
                        q1 = pffn.tile([128, hcol], f32, tag="wk_a")
                        nc.vector.tensor_scalar(q1[:], wc[:], rs_b[:, 0:1],
                                                MAGIC, A.mult, A.add)
                        q2 = pffn.tile([128, hcol], f32, tag="wk_b")
                        nc.vector.tensor_scalar(q2[:], q1[:], MAGIC - 1.0, 2.0,
                                                A.subtract, A.min)
                        nc.vector.tensor_scalar(
                            out_tile[:, ck // NSP,
                                     (ck % NSP) * hcol:(ck % NSP + 1) * hcol],
                            q2[:], 0.0, None, A.max)
                    return s_b

                gq = pwq.tile([128, DC, FFN], fp8, tag="gq")
                uq = pwq.tile([128, DC, FFN], fp8, tag="uq")
                dq = pwq.tile([128, KC, D], bf16, tag="dq")
                sg_b = quant_weight(gwT_d, D, FFN, gq, fp8)
                su_b = quant_weight(uwT_d, D, FFN, uq, fp8)
                sd_b = quant_weight(dwT_d, FFN, D, dq, bf16)

                # ---------- gather + int4 quant ----------
                n0_reg = nc.gpsimd.alloc_register("n0reg")
                xg = phb.tile([128, NT, D], f32, tag="xgy")
                dgsem = nc.alloc_semaphore("dgsem")
                with tc.tile_critical():
                    nc.gpsimd.load(n0_reg, n0i[0:1, 0:1])
                    nc.gpsimd.dma_gather(
                        xg[:], x_d[:, :], idx16[:], 640, n0_reg, D
                    ).then_inc(dgsem, 16)
                    nc.gpsimd.wait_ge(dgsem, 16)
                amax5 = psm.tile([128, NT], f32)
                nc.vector.tensor_reduce(amax5[:], xg[:], axis=X, op=A.max,
                                        apply_absolute_value=True)
                r5 = psm.tile([128, NT], f32)
                nc.vector.reciprocal(r5[:], amax5[:])
                s7 = psm.tile([128, NT], f32)
                nc.vector.tensor_scalar_mul(s7[:], r5[:], 7.0)
                xq = pffn.tile([128, NT, D], bf16, tag="xq")
                sumxq = psm.tile([128, NT], f32)
                for t in range(NT):
                    xv1 = pffn.tile([128, D], f32, tag="xv1")
                    nc.vector.tensor_scalar(xv1[:], xg[:, t, :], s7[:, t:t + 1],
                                            MAGIC, A.mult, A.add)
                    nc.vector.tensor_scalar(xq[:, t, :], xv1[:], MAGIC, 0.0,
                                            A.subtract, A.add,
                                            accum_out=sumxq[:, t:t + 1])
                xqT = pffn.tile([128, NT, DC, 128], fp8, tag="xqT")
                for t in range(NT):
                    for c in range(DC):
                        xtt = pffn.tile([128, 128], bf16, tag="xtt")
                        nc.sync.dma_start_transpose(
                            xtt[:], xq[:, t, c * 128:(c + 1) * 128])
                        nc.vector.tensor_copy(xqT[:, t, c, :], xtt[:])

                gsc = psm.tile([128, NT], f32)
                nc.vector.tensor_tensor(gsc[:], amax5[:],
                                        sg_b[:, 0:1].broadcast_to((128, NT)),
                                        A.mult)
                nc.vector.tensor_scalar_mul(gsc[:], gsc[:], 1.0 / 7.0)
                gbias = psm.tile([128, NT], f32)
                nc.vector.tensor_tensor(gbias[:], sumxq[:], gsc[:], A.mult)
                nc.vector.tensor_scalar_mul(gbias[:], gbias[:], -1.0)
                usc = psm.tile([128, NT], f32)
                nc.vector.tensor_tensor(usc[:], amax5[:],
                                        su_b[:, 0:1].broadcast_to((128, NT)),
                                        A.mult)
                nc.vector.tensor_scalar_mul(usc[:], usc[:], 1.0 / 7.0)

                yout = phb.tile([128, NT, D], f32, tag="xgy")
                act_fn = AF.Sigmoid if sim_sigmoid else AF.Silu

                for t in range(NT):
                    par = str(t % 2)
                    htil = phb.tile([128, FFN], f32, tag="htil" + par)
                    for fc in range(FC):
                        g_ps = ppf.tile([128, 512], f32, tag="gps")
                        u_ps = ppf.tile([128, 512], f32, tag="ups")
                        for c in range(DC):
                            nc.tensor.matmul(
                                g_ps[:], xqT[:, t, c, :],
                                gq[:, c, fc * 512:(fc + 1) * 512],
                                start=(c == 0), stop=(c == DC - 1))
                        for c in range(DC):
                            nc.tensor.matmul(
                                u_ps[:], xqT[:, t, c, :],
                                uq[:, c, fc * 512:(fc + 1) * 512],
                                start=(c == 0), stop=(c == DC - 1))
                        sgl = pffn.tile([128, 512], f32, tag="sgl")
                        nc.scalar.activation(
                            sgl[:], g_ps[:], act_fn,
                            bias=gbias[:, t:t + 1], scale=gsc[:, t:t + 1])
                        ucc = pffn.tile([128, 512], f32, tag="ucc")
                        nc.vector.tensor_scalar(
                            ucc[:], u_ps[:], sumxq[:, t:t + 1], None,
                            A.subtract)
                        nc.vector.tensor_tensor(
                            htil[:, fc * 512:(fc + 1) * 512], sgl[:], ucc[:],
                            A.mult)
                    amaxh = psm.tile([128, 1], f32, tag="amaxh" + par)
                    nc.vector.tensor_reduce(amaxh[:], htil[:], axis=X, op=A.max,
                                            apply_absolute_value=True)
                    lo = psm.tile([128, 1], f32, tag="lo" + par)
                    nc.vector.memset(lo[:], 0.0)
                    hi = psm.tile([128, 1], f32, tag="hi" + par)
                    nc.vector.tensor_copy(hi[:], amaxh[:])
                    mid = psm.tile([128, 1], f32, tag="mid" + par)
                    nmid = psm.tile([128, 1], f32, tag="nmid" + par)
                    cnta = psm.tile([128, 1], f32, tag="cnta" + par)
                    sgn = psm.tile([128, 1], f32, tag="sgn" + par)
                    c2 = psm.tile([128, 1], f32, tag="c2" + par)
                    junk = pffn.tile([128, FFN], bf16, tag="hq")
                    junka = pffn.tile([128, FFN], bf16, tag="msk")
                    # count(h >= mid) on DVE; count(h <= -mid) via ACT
                    # sign-accumulate: sum sign(-h - mid) = #lt - #gt, so
                    # cntb ~= (FFN + sum)/2 (ties measure-zero). The keep
                    # test cnta + cntb >= KTH becomes
                    # cnta + sum/2 >= KTH - FFN/2.
                    for it in range(SEARCH_ITERS):
                        nc.vector.tensor_add(mid[:], lo[:], hi[:])
                        nc.vector.tensor_scalar_mul(mid[:], mid[:], 0.5)
                        nc.vector.tensor_scalar_mul(nmid[:], mid[:], -1.0)
                        nc.vector.tensor_scalar(
                            junk[:], htil[:], mid[:, 0:1], 0.0,
                            A.is_ge, A.add, accum_out=cnta[:])
                        nc.scalar.activation(
                            junka[:], htil[:], AF.Sign,
                            bias=nmid[:, 0:1], scale=-1.0, accum_out=sgn[:])
                        nc.vector.scalar_tensor_tensor(
                            c2[:], sgn[:], 0.5, cnta[:], A.mult, A.add)
                        ge = psm.tile([128, 1], mybir.dt.uint8, tag="ge" + par)
                        nc.vector.tensor_scalar(ge[:], c2[:],
                                                float(KTH - FFN // 2),
                                                None, A.is_ge)
                        nc.vector.copy_predicated(lo[:], ge[:], mid[:])
                        gei = psm.tile([128, 1], mybir.dt.uint8, tag="gei" + par)
                        nc.vector.tensor_scalar(gei[:], c2[:],
                                                float(KTH - FFN // 2),
                                                None, A.is_lt)
                        nc.vector.copy_predicated(hi[:], gei[:], mid[:])
                    s8r = psm.tile([128, 1], f32, tag="s8r" + par)
                    nc.vector.reciprocal(s8r[:], amaxh[:])
                    s8 = psm.tile([128, 1], f32, tag="s8" + par)
                    nc.vector.tensor_scalar_mul(s8[:], s8r[:], 127.0)
                    nlo = psm.tile([128, 1], f32, tag="nmid" + par)
                    nc.vector.tensor_scalar_mul(nlo[:], lo[:], -1.0)
                    mska = pffn.tile([128, FFN], bf16, tag="hqT")
                    nc.vector.tensor_scalar(mska[:], htil[:], lo[:, 0:1], None,
                                            A.is_ge)
                    msk = pffn.tile([128, FFN], bf16, tag="msk")
                    nc.vector.tensor_scalar(msk[:], htil[:], nlo[:, 0:1], 1.0,
                                            A.is_le, A.mult)
                    nc.vector.tensor_add(msk[:], msk[:], mska[:])
                    nc.vector.tensor_scalar(htil[:], htil[:], s8[:, 0:1], MAGIC,
                                            A.mult, A.add)
                    hq = pffn.tile([128, FFN], bf16, tag="hq")
                    sumq = psm.tile([128, 1], f32, tag="sumq" + par)
                    nc.vector.scalar_tensor_tensor(
                        hq[:], htil[:], MAGIC, msk[:], A.subtract, A.mult,
                        accum_out=sumq[:])
                    hqT = pffn.tile([128, KC, 128], bf16, tag="hqT")
                    for c in range(KC):
                        nc.sync.dma_start_transpose(
                            hqT[:, c, :], hq[:, c * 128:(c + 1) * 128])
                    y_ps0 = ppy.tile([128, 384], f32, tag="yps0")
                    y_ps1 = ppy.tile([128, 384], f32, tag="yps1")
                    y_ps = [y_ps0, y_ps1]
                    for j in range(2):
                        for c in range(KC):
                            nc.tensor.matmul(
                                y_ps[j][:], hqT[:, c, :],
                                dq[:, c, j * 384:(j + 1) * 384],
                                start=(c == 0), stop=(c == KC - 1))
                    fsc = psm.tile([128, 1], f32, tag="fsc" + par)
                    nc.vector.tensor_tensor(fsc[:], amaxh[:], usc[:, t:t + 1],
                                            A.mult)
                    nc.vector.tensor_tensor(fsc[:], fsc[:], sd_b[:, 0:1],
                                            A.mult)
                    nc.vector.tensor_tensor(fsc[:], fsc[:], cw_tl[:, t:t + 1],
                                            A.mult)
                    nc.vector.tensor_scalar_mul(fsc[:], fsc[:], 1.0 / 127.0)
                    for j in range(2):
                        nc.vector.tensor_scalar(
                            yout[:, t, j * 384:(j + 1) * 384], y_ps[j][:],
                            sumq[:, 0:1], fsc[:, 0:1], A.subtract, A.mult)

                scsem = nc.alloc_semaphore("scsem")
                with tc.tile_critical():
                    nc.gpsimd.dma_scatter_add(
                        out_d[:, :], yout[:], idx16[:], 640, n0_reg, D
                    ).then_inc(scsem, 16)
                    nc.gpsimd.wait_ge(scsem, 16)

    nc.compile()
    return nc


_NC_CACHE = {}


def _get_nc(sim_sigmoid=False):
    key = bool(sim_sigmoid)
    if key not in _NC_CACHE:
        _NC_CACHE[key] = build(sim_sigmoid=key)
    return _NC_CACHE[key]


def make_in_maps(x, gate_w, up_w, down_w, router_w):
    x2 = np.ascontiguousarray(np.asarray(x).reshape(BT, D).astype(np.float32))
    xT = np.ascontiguousarray(x2.T)
    rw = np.ascontiguousarray(np.asarray(router_w).astype(np.float32))
    in_maps = []
    for e in range(NE):
        in_maps.append({
            "x": x2,
            "xT": xT,
            "gwT": np.ascontiguousarray(np.asarray(gate_w[e]).T.astype(np.float32)),
            "uwT": np.ascontiguousarray(np.asarray(up_w[e]).T.astype(np.float32)),
            "dwT": np.ascontiguousarray(np.asarray(down_w[e]).T.astype(np.float32)),
            "rw": rw,
            "eid": np.array([[float(e)]], dtype=np.float32),
        })
    return in_maps


def kernel(x, gate_w, up_w, down_w, router_w):
    from concourse.bass_utils import run_bass_kernel_spmd

    nc = _get_nc(sim_sigmoid=False)
    in_maps = make_in_maps(x, gate_w, up_w, down_w, router_w)
    res = run_bass_kernel_spmd(nc, in_maps, core_ids=list(range(NE)))
    out = np.zeros((BT, D), np.float32)
    for e in range(NE):
        out += res.results[e]["out"]
    aux = np.float32(res.results[0]["aux"][0, 0])
    return out.reshape(B, T, D), aux
